# revision 21
# baseline (speedup 1.0000x reference)
"""MLA (DeepSeek-style multi-head latent attention) forward on 8 TRN2 NeuronCores.

Sharding: the q_down projection (the largest replicated GEMM) is sharded over
sequence (each core computes its own 256 rows, normalized, bf16) and
AllGathered on-device while the still-replicated ckv projection and the K/V
materialization run on the tensor engine — the collective hides behind ~93us
of PE work. Attention and the output projection are tensor-parallel over
heads (16 heads -> 2 per core); partial wo outputs are summed on host.

Device layout is "feature-major" (features on SBUF partitions, sequence on the
free dim) throughout. Attention uses the prefill-optimal NON-absorbed form:
per-head K (128-dim nope) and V (128-dim) are materialized from the shared
latent once, so scores contract over 192 dims (not 576) and ctx over 128
(not 512). Scores come out k-major ([k, q]); softmax normalization over k is
done with ones-matmuls on the tensor engine.

The projections run with bf16 inputs and weights (half the DMA, same PE rate
as fp32r at free-dim >= 256), accumulating in fp32 PSUM. Attention operands
stay float32r (TF32). wo partials return in bf16 and are summed in fp32 on
host.

Pipeline per core (S=2048; 4 seq-chunks of 512 for full-S phases):
  L:  local q_down shard: q_downT[:, own 256 cols] = wq_a.T^T @ hid_ownT,
      software-pipelined li-groups (sum-of-squares via ACT Square straight
      from PSUM + ones-matmul trail one group behind); rms fold: shard is
      normalized (r_q broadcast via gpsimd) and rounded to bf16; DMA to DRAM
      -> AllGather across the 8 cores.
  C:  ckvT = wkv_a'.T^T @ hidT replicated (RoPE interleave baked into the pe
      rows of wkv_a) -> DRAM spill. Runs while the AllGather flies.
  K:  two-stage pipeline per chunk: latent rms (kv ln folded into the
      broadcast matmul), then per-head k_nopeT (feature-major) and V
      (seq-major, both heads side by side) for the previous chunk; RoPE k_pe.
  B:  post-gather wq_b: qT (all heads' rows for this core) from the gathered
      normalized q_down, PSUM-accumulated over the 12 l-tiles; RoPE q_pe.
  A:  per k-block, software-pipelined over both heads: scoresT -> exp (no
      max subtraction needed: |score*scale| <= ~4) -> causal mask on the
      diagonal block (suffix-sliced matmuls skip fully-masked columns) ->
      ctxT + softmax denominator accumulation in PSUM; per-head epilogues
      deferred into the next chunk's score stream.
  W:  wo partial matmul -> bf16 DRAM outT.
Host: sum the 8 partial outT in fp32, transpose -> [1, S, HID].
"""

import numpy as np

S = 2048
HID = 2048
QLR = 1536
H_PER_CORE = 2
N_CORES = 8
NOPE = 128
ROPE = 64
VD = 128
KVL = 512
EPS = 1e-6
THETA = 10000.0
SCALE = float((NOPE + ROPE) ** -0.5)
NC_ = 4            # seq chunks
CW = 512           # chunk width
SHW = S // N_CORES  # 256-wide local shard
KB = S // 128      # 16 k-blocks
NLT = QLR // 128   # 12 l-tiles


def _tf32_rne(a):
    a = np.ascontiguousarray(a, dtype=np.float32)
    u = a.view(np.uint32).astype(np.uint64)
    u = (u + 0xFFF + ((u >> 13) & 1)) & 0xFFFFE000
    return u.astype(np.uint32).view(np.float32)


def _bf16(a):
    import ml_dtypes
    return np.ascontiguousarray(np.asarray(a, np.float32)).astype(ml_dtypes.bfloat16)


def _build_program():
    import concourse.mybir as mybir
    import concourse.tile as tile
    from concourse import bacc

    f32 = mybir.dt.float32
    f32r = mybir.dt.float32r
    bf16 = mybir.dt.bfloat16
    AF = mybir.ActivationFunctionType
    OP = mybir.AluOpType

    nc = bacc.Bacc("TRN2", target_bir_lowering=False, num_devices=N_CORES)

    hid_own = nc.dram_tensor("hid_own", [HID, SHW], bf16, kind="ExternalInput")
    hid_t = nc.dram_tensor("hid_t", [HID, S], bf16, kind="ExternalInput")
    wqa_t = nc.dram_tensor("wqa_t", [HID, QLR], bf16, kind="ExternalInput")
    wqb_t = nc.dram_tensor("wqb_t", [QLR, 384], bf16, kind="ExternalInput")
    wkv_t = nc.dram_tensor("wkv_t", [HID, 640], bf16, kind="ExternalInput")
    kvln_d = nc.dram_tensor("kvln", [1, KVL], f32r, kind="ExternalInput")
    wukt_d = nc.dram_tensor("wukt", [H_PER_CORE, KVL, NOPE], f32r,
                            kind="ExternalInput")
    wuv2_d = nc.dram_tensor("wuv2", [KVL, H_PER_CORE * VD], f32r,
                            kind="ExternalInput")
    wo_t = nc.dram_tensor("wo_t", [H_PER_CORE * VD, HID], f32r, kind="ExternalInput")
    cos2_d = nc.dram_tensor("cos2", [128, S], f32, kind="ExternalInput")
    sin2n_d = nc.dram_tensor("sin2n", [128, S], f32, kind="ExternalInput")
    swapp_d = nc.dram_tensor("swapp", [128, 128], f32r, kind="ExternalInput")
    maskt_d = nc.dram_tensor("maskt", [128, 128], f32, kind="ExternalInput")
    out_t = nc.dram_tensor("out_t", [HID, S], bf16, kind="ExternalOutput")

    with tile.TileContext(nc) as tc:
        with (
            tc.tile_pool(name="stats", bufs=1) as stats,
            tc.tile_pool(name="dram", bufs=1, space="DRAM") as dram,
        ):
            ones_p = stats.tile([128, 1], f32r)
            nc.vector.memset(ones_p.bitcast(f32), 1.0)
            ones_row = stats.tile([1, 128], f32r)
            nc.vector.memset(ones_row.bitcast(f32), 1.0)
            eps_sb = stats.tile([1, 1], f32)
            nc.vector.memset(eps_sb, EPS)

            ckv_spill = dram.tile([128, 5, S], f32r)
            qdn_shard = dram.tile([QLR, SHW], bf16)
            qdn_full = dram.tile([N_CORES * QLR, SHW], bf16)

            # ========== Phase L: local q_down shard -> AllGather ==========
            with (
                tc.tile_pool(name="hidop", bufs=1) as hidop,
                tc.tile_pool(name="wqap", bufs=1) as wqap,
                tc.tile_pool(name="lwork", bufs=2) as lwork,
                tc.tile_pool(name="lbig", bufs=1) as lbig,
                tc.tile_pool(name="lpsum", bufs=2, space="PSUM") as lpsum,
                tc.tile_pool(name="lpsum1", bufs=1, space="PSUM") as lpsum1,
            ):
                hid_own_sb = hidop.tile([128, KB, SHW], bf16)
                for kt in range(KB):
                    nc.sync.dma_start(
                        hid_own_sb[:, kt, :],
                        hid_own.ap()[128 * kt : 128 * (kt + 1), :],
                    )
                wqa_sb = wqap.tile([128, KB, QLR], bf16)
                for lg in range(3):
                    nc.sync.dma_start(
                        wqa_sb[:, :, 512 * lg : 512 * (lg + 1)],
                        wqa_t.ap()[:, 512 * lg : 512 * (lg + 1)].rearrange(
                            "(kt p) m -> p kt m", p=128
                        ),
                    )
                qdraw = lbig.tile([128, NLT, SHW], f32)
                qdn_sb = lbig.tile([128, NLT, SHW], bf16)
                ssql = stats.tile([1, SHW], f32)
                rqbl = stats.tile([128, SHW], f32)
                ps_ssq = lpsum1.tile([1, SHW], f32, tag="ssq")

                prev_qd = [None]

                def emit_ssq(li):
                    sq = lwork.tile([128, SHW], f32r, tag="sq")
                    nc.scalar.activation(sq, prev_qd[0], AF.Square)
                    nc.tensor.matmul(
                        ps_ssq, ones_p, sq, start=(li == 0), stop=(li == NLT - 1)
                    )

                for li in range(NLT):
                    ps_qd = lpsum.tile([128, SHW], f32, tag="qd",
                                       name=f"ps_qd{li % 2}")
                    for kt in range(KB):
                        nc.tensor.matmul(
                            ps_qd,
                            wqa_sb[:, kt, 128 * li : 128 * (li + 1)],
                            hid_own_sb[:, kt, :],
                            start=(kt == 0),
                            stop=(kt == KB - 1),
                        )
                    if li > 0:
                        emit_ssq(li - 1)
                    nc.vector.tensor_copy(qdraw[:, li, :], ps_qd)
                    prev_qd[0] = ps_qd
                emit_ssq(NLT - 1)
                nc.scalar.activation(
                    ssql, ps_ssq, AF.Sqrt, scale=1.0 / QLR, bias=eps_sb
                )
                nc.vector.reciprocal_approx_fast(out=ssql, in_=ssql)
                nc.gpsimd.partition_broadcast(rqbl, ssql, channels=128)
                for li in range(NLT):
                    nc.vector.tensor_tensor(
                        qdn_sb[:, li, :], qdraw[:, li, :], rqbl, OP.mult
                    )
                nc.gpsimd.dma_start(
                    qdn_shard.rearrange("(li p) s -> p li s", p=128),
                    qdn_sb,
                )
                nc.gpsimd.collective_compute(
                    "AllGather",
                    mybir.AluOpType.bypass,
                    replica_groups=[list(range(N_CORES))],
                    ins=[qdn_shard.opt()],
                    outs=[qdn_full.opt()],
                )

            # ========== Phase C: replicated ckvT (overlaps the AllGather) =====
            with (
                tc.tile_pool(name="hidp", bufs=1) as hidp,
                tc.tile_pool(name="wkvp", bufs=1) as wkvp,
                tc.tile_pool(name="cwork", bufs=2) as cwork,
                tc.tile_pool(name="cpsum", bufs=2, space="PSUM") as cpsum,
            ):
                hid_sb = hidp.tile([128, KB, S], bf16)
                for kt in range(KB):
                    nc.sync.dma_start(
                        hid_sb[:, kt, :], hid_t.ap()[128 * kt : 128 * (kt + 1), :]
                    )
                wkv_sb = wkvp.tile([128, KB, 640], bf16, tag="wkv")
                for dt in range(5):
                    nc.sync.dma_start(
                        wkv_sb[:, :, 128 * dt : 128 * (dt + 1)],
                        wkv_t.ap()[:, 128 * dt : 128 * (dt + 1)].rearrange(
                            "(kt p) m -> p kt m", p=128
                        ),
                    )
                for c in range(NC_):
                    cs = slice(CW * c, CW * (c + 1))
                    for dt in range(5):
                        ps_ck = cpsum.tile([128, CW], f32, tag="ck")
                        for kt in range(KB):
                            nc.tensor.matmul(
                                ps_ck,
                                wkv_sb[:, kt, 128 * dt : 128 * (dt + 1)],
                                hid_sb[:, kt, cs],
                                start=(kt == 0),
                                stop=(kt == KB - 1),
                            )
                        ckb = cwork.tile([128, CW], f32r, tag="ckb")
                        nc.vector.tensor_copy(ckb, ps_ck)
                        nc.sync.dma_start(ckv_spill[:, dt, cs], ckb)

            # ============ late constants + persistent attention tensors ============
            with (
                tc.tile_pool(name="consts", bufs=1) as consts,
                tc.tile_pool(name="resid", bufs=1) as resid,
            ):
                kvln_sb = consts.tile([1, KVL], f32r)
                nc.sync.dma_start(kvln_sb, kvln_d.ap())
                wukt_sb = consts.tile([128, H_PER_CORE, 4, NOPE], f32r)
                nc.sync.dma_start(
                    wukt_sb, wukt_d.ap().rearrange("h (lt p) n -> p h lt n", p=128)
                )
                wuv2_sb = consts.tile([128, 4, H_PER_CORE * VD], f32r)
                nc.sync.dma_start(
                    wuv2_sb, wuv2_d.ap().rearrange("(lt p) v -> p lt v", p=128)
                )
                cos2_sb = consts.tile([128, S], f32)
                nc.sync.dma_start(cos2_sb, cos2_d.ap())
                sin2n_sb = consts.tile([128, S], f32)
                nc.sync.dma_start(sin2n_sb, sin2n_d.ap())
                swapp_sb = consts.tile([128, 128], f32r)
                nc.sync.dma_start(swapp_sb, swapp_d.ap())
                maskt_sb = consts.tile([128, 128], f32)
                nc.sync.dma_start(maskt_sb, maskt_d.ap())

                kpe = resid.tile([128, S], f32r)          # roped k_peT (2 head copies)
                kn_sb = resid.tile([128, H_PER_CORE, S], f32r)  # per-head k_nopeT
                v_sb = resid.tile([128, KB, H_PER_CORE * VD], f32r)  # V seq-major
                ctxa = resid.tile([128, H_PER_CORE, S], f32r)
                wo_sb = resid.tile([128, H_PER_CORE, HID], f32r)
                nc.sync.dma_start(
                    wo_sb, wo_t.ap().rearrange("(h p) m -> p h m", p=128)
                )
                qtr = resid.tile([128, 3, S], f32r)       # post-gather q (r_q folded)
                qspe = resid.tile([128, S], f32r)         # roped q_peT

                # ===== Phase K: latent rms + per-head K/V materialization =====
                # Two-stage pipeline: rms chain of chunk c overlaps the
                # materialization matmuls of chunk c-1 on the PE.
                with (
                    tc.tile_pool(name="kwork", bufs=2) as kwork,
                    tc.tile_pool(name="kpsum", bufs=2, space="PSUM") as kpsum,
                    tc.tile_pool(name="kpsum1", bufs=1, space="PSUM") as kpsum1,
                ):
                    ksn_tiles = [None] * NC_
                    ck_tiles = [None] * NC_
                    rk_tiles = [None] * NC_

                    def emit_rms_a(c):
                        cs = slice(CW * c, CW * (c + 1))
                        ck = kwork.tile([128, 5, CW], f32r, tag="ck",
                                        name=f"ck{c}")
                        nc.sync.dma_start(ck, ckv_spill[:, :, cs])
                        ps_ssqk = kpsum1.tile([1, CW], f32, tag="ssqk")
                        for j in range(4):
                            sqk = kwork.tile([128, CW], f32r, tag="sqk")
                            nc.scalar.activation(
                                sqk, ck[:, j, :].bitcast(f32), AF.Square
                            )
                            nc.tensor.matmul(
                                ps_ssqk, ones_p, sqk, start=(j == 0), stop=(j == 3)
                            )
                        rk = kwork.tile([1, CW], f32, tag="rk")
                        nc.scalar.activation(
                            rk, ps_ssqk, AF.Sqrt, scale=1.0 / KVL, bias=eps_sb
                        )
                        nc.vector.reciprocal_approx_fast(out=rk, in_=rk)
                        rk_r = kwork.tile([1, CW], f32r, tag="rkr")
                        nc.vector.tensor_copy(rk_r, rk)
                        ck_tiles[c] = ck
                        rk_tiles[c] = rk_r

                    def emit_rms_b(c):
                        ck = ck_tiles[c]
                        rk_r = rk_tiles[c]
                        ksn_c = kwork.tile([128, 4, CW], f32r, tag="ksn",
                                           name=f"ksn{c}")
                        for j in range(4):
                            ps_b = kpsum1.tile([128, CW], f32, tag="bc")
                            nc.tensor.matmul(
                                ps_b,
                                kvln_sb[0:1, 128 * j : 128 * (j + 1)],
                                rk_r,
                                start=True,
                                stop=True,
                            )
                            nc.vector.tensor_tensor(
                                ksn_c[:, j, :], ck[:, j, :].bitcast(f32), ps_b,
                                OP.mult
                            )
                        ksn_tiles[c] = ksn_c

                    def emit_mat(c):
                        cs = slice(CW * c, CW * (c + 1))
                        ksn_c = ksn_tiles[c]
                        ck = ck_tiles[c]
                        for h in range(H_PER_CORE):
                            ps_k = kpsum.tile([128, CW], f32, tag="kn")
                            for lt in range(4):
                                nc.tensor.matmul(
                                    ps_k,
                                    wukt_sb[:, h, lt, :],
                                    ksn_c[:, lt, :],
                                    start=(lt == 0),
                                    stop=(lt == 3),
                                )
                            nc.vector.tensor_copy(kn_sb[:, h, cs], ps_k)
                        for b in range(4):
                            ps_v = kpsum.tile([128, H_PER_CORE * VD], f32, tag="v")
                            for lt in range(4):
                                nc.tensor.matmul(
                                    ps_v,
                                    ksn_c[:, lt, 128 * b : 128 * (b + 1)],
                                    wuv2_sb[:, lt, :],
                                    start=(lt == 0),
                                    stop=(lt == 3),
                                )
                            nc.vector.tensor_copy(v_sb[:, 4 * c + b, :], ps_v)
                        # k_pe rope (both 64-row copies at once)
                        ps_sw = kpsum1.tile([128, CW], f32, tag="sw")
                        nc.tensor.matmul(
                            ps_sw, swapp_sb, ck[:, 4, :], start=True, stop=True
                        )
                        t1 = kwork.tile([128, CW], f32, tag="t1")
                        nc.vector.tensor_tensor(
                            t1, ck[:, 4, :].bitcast(f32), cos2_sb[:, cs], OP.mult
                        )
                        t2 = kwork.tile([128, CW], f32, tag="t2")
                        nc.vector.tensor_tensor(t2, ps_sw, sin2n_sb[:, cs], OP.mult)
                        nc.vector.tensor_tensor(kpe[:, cs], t1, t2, OP.add)

                    emit_rms_a(0)
                    emit_rms_b(0)
                    for c in range(1, NC_):
                        emit_rms_a(c)
                        emit_mat(c - 1)
                        emit_rms_b(c)
                    emit_mat(NC_ - 1)

                # ===== Phase B: post-gather wq_b + q rope =====
                with (
                    tc.tile_pool(name="bqp", bufs=1) as bqp,
                    tc.tile_pool(name="bwork", bufs=2) as bwork,
                    tc.tile_pool(name="bpsum", bufs=2, space="PSUM") as bpsum,
                    tc.tile_pool(name="bmisc", bufs=1, space="PSUM") as bmisc,
                ):
                    qdn_all = bqp.tile([128, NLT, N_CORES, SHW], bf16)
                    qdn_src = qdn_full.rearrange(
                        "(r li p) s -> p li r s", p=128, li=NLT
                    )
                    for li in range(NLT):
                        nc.gpsimd.dma_start(
                            qdn_all[:, li, :, :], qdn_src[:, li, :, :]
                        )
                    wqb_sb = bqp.tile([128, NLT, 384], bf16)
                    nc.sync.dma_start(
                        wqb_sb,
                        wqb_t.ap().rearrange("(li p) m -> p li m", p=128),
                    )
                    for c in range(NC_):
                        cs = slice(CW * c, CW * (c + 1))
                        for dt in range(3):
                            ps_qt = bpsum.tile([128, CW], f32, tag="qt")
                            for li in range(NLT):
                                nc.tensor.matmul(
                                    ps_qt,
                                    wqb_sb[:, li, 128 * dt : 128 * (dt + 1)],
                                    qdn_all[:, li, 2 * c : 2 * c + 2, :],
                                    start=(li == 0),
                                    stop=(li == NLT - 1),
                                )
                            nc.vector.tensor_copy(qtr[:, dt, cs], ps_qt)
                        # rope q_pe (both heads stacked)
                        ps_sw = bmisc.tile([128, CW], f32, tag="misc",
                                           name="ps_swq")
                        nc.tensor.matmul(
                            ps_sw, swapp_sb, qtr[:, 2, cs], start=True, stop=True
                        )
                        t1 = bwork.tile([128, CW], f32, tag="t1")
                        nc.vector.tensor_tensor(
                            t1, qtr[:, 2, cs].bitcast(f32), cos2_sb[:, cs], OP.mult
                        )
                        t2 = bwork.tile([128, CW], f32, tag="t2")
                        nc.vector.tensor_tensor(t2, ps_sw, sin2n_sb[:, cs], OP.mult)
                        nc.vector.tensor_tensor(qspe[:, cs], t1, t2, OP.add)

                # ================= Phase A: attention =================
                with (
                    tc.tile_pool(name="asm", bufs=2) as asm,
                    tc.tile_pool(name="attp", bufs=3) as attp,
                    tc.tile_pool(name="aacc", bufs=1, space="PSUM") as aacc,
                    tc.tile_pool(name="ascore", bufs=2, space="PSUM") as ascore,
                    tc.tile_pool(name="ascr1", bufs=1, space="PSUM") as ascr1,
                    tc.tile_pool(name="amisc", bufs=1, space="PSUM") as amisc,
                ):
                    pending_epilogue = [None]

                    for c in range(NC_):
                        cs = slice(CW * c, CW * (c + 1))
                        nj = 4 * c + 4
                        ps_ctx = [
                            aacc.tile([128, CW], f32, tag=f"ctx{h}",
                                      name=f"ps_ctx{h}")
                            for h in range(H_PER_CORE)
                        ]
                        ps_sum = [
                            aacc.tile([1, CW], f32, tag=f"sum{h}",
                                      name=f"ps_sum{h}")
                            for h in range(H_PER_CORE)
                        ]
                        # software pipeline: scores(j) on PE, then ctx(j-1);
                        # exp(j) on ACT overlaps ctx(j-1)+scores(j+1) on PE.
                        atts = [[None, None] for _ in range(nj)]
                        offs = [0 if j < 4 * c else 128 * (j - 4 * c)
                                for j in range(nj)]

                        def emit_scores(j, c=c, nj=nj, offs=offs, atts=atts):
                            off = offs[j]
                            q0 = CW * c + off
                            qs = slice(q0, CW * (c + 1))
                            for h in range(H_PER_CORE):
                                pool = ascore if h == 0 else ascr1
                                ps_s = pool.tile([128, CW], f32, tag=f"sc{h}",
                                                 name=f"ps_s{h}")
                                nc.tensor.matmul(
                                    ps_s[:, off:],
                                    kn_sb[:, h, 128 * j : 128 * (j + 1)],
                                    qtr[:, h, qs],
                                    start=True,
                                    stop=False,
                                )
                                nc.tensor.matmul(
                                    ps_s[:, off:],
                                    kpe[64 * h : 64 * (h + 1),
                                        128 * j : 128 * (j + 1)],
                                    qspe[64 * h : 64 * (h + 1), qs],
                                    start=False,
                                    stop=True,
                                )
                                att = attp.tile([128, CW], f32r, tag=f"att{h}",
                                                name=f"att{h}")
                                nc.scalar.activation(
                                    att[:, off:], ps_s[:, off:], AF.Exp, scale=SCALE
                                )
                                if j >= 4 * c:
                                    nc.vector.tensor_tensor(
                                        att[:, off : off + 128],
                                        att[:, off : off + 128].bitcast(f32),
                                        maskt_sb,
                                        OP.mult,
                                    )
                                atts[j][h] = att

                        def emit_ctx(j, c=c, nj=nj, offs=offs, atts=atts,
                                     ps_ctx=ps_ctx, ps_sum=ps_sum):
                            off = offs[j]
                            for h in range(H_PER_CORE):
                                nc.tensor.matmul(
                                    ps_ctx[h][:, off:],
                                    v_sb[:, j, VD * h : VD * (h + 1)],
                                    atts[j][h][:, off:],
                                    start=(j == 0),
                                    stop=(j == nj - 1),
                                )
                                nc.tensor.matmul(
                                    ps_sum[h][:, off:],
                                    ones_p,
                                    atts[j][h][:, off:],
                                    start=(j == 0),
                                    stop=(j == nj - 1),
                                )

                        emit_scores(0)
                        emit_scores(1)
                        if pending_epilogue[0] is not None:
                            pending_epilogue[0]()
                        emit_ctx(0)
                        for j in range(2, nj):
                            emit_scores(j)
                            emit_ctx(j - 1)
                        emit_ctx(nj - 1)

                        def epilogue(c=c, cs=cs, ps_ctx=ps_ctx, ps_sum=ps_sum):
                            for h in range(H_PER_CORE):
                                s_sb = asm.tile([1, CW], f32, tag="ssb")
                                nc.vector.tensor_copy(s_sb, ps_sum[h])
                                nc.vector.reciprocal_approx_fast(
                                    out=s_sb, in_=s_sb
                                )
                                rs_r = asm.tile([1, CW], f32r, tag="rsr")
                                nc.vector.tensor_copy(rs_r, s_sb)
                                ps_rb2 = amisc.tile([128, CW], f32, tag="misc",
                                                    name="ps_rb2")
                                nc.tensor.matmul(
                                    ps_rb2, ones_row, rs_r, start=True, stop=True
                                )
                                rsb = asm.tile([128, CW], f32, tag="rsb")
                                nc.vector.tensor_copy(rsb, ps_rb2)
                                nc.vector.tensor_tensor(
                                    ctxa[:, h, cs], ps_ctx[h], rsb, OP.mult
                                )

                        pending_epilogue[0] = epilogue
                    pending_epilogue[0]()

                # ================= Phase W: output projection =================
                with (
                    tc.tile_pool(name="obp", bufs=3) as obp,
                    tc.tile_pool(name="wpsum", bufs=2, space="PSUM") as wpsum,
                ):
                    for c in range(NC_):
                        cs = slice(CW * c, CW * (c + 1))
                        for ht in range(KB):
                            ps_o = wpsum.tile([128, CW], f32, tag="o")
                            for h in range(H_PER_CORE):
                                nc.tensor.matmul(
                                    ps_o,
                                    wo_sb[:, h, 128 * ht : 128 * (ht + 1)],
                                    ctxa[:, h, cs],
                                    start=(h == 0),
                                    stop=(h == H_PER_CORE - 1),
                                )
                            ob = obp.tile([128, CW], bf16, tag="ob")
                            nc.vector.tensor_copy(ob, ps_o)
                            nc.sync.dma_start(
                                out_t.ap()[128 * ht : 128 * (ht + 1), cs], ob
                            )

    nc.finalize()
    return nc


_PROGRAM = None


def _get_program():
    global _PROGRAM
    if _PROGRAM is None:
        _PROGRAM = _build_program()
    return _PROGRAM


def _host_inputs(hidden_states, position_ids, wq_a, q_a_ln_w, wq_b, wkv_a,
                 kv_a_ln_w, wkv_b, wo):
    """Build the 8 per-core input maps."""
    hs = np.asarray(hidden_states, np.float32)[0]          # [S, HID]
    pos = np.asarray(position_ids)[0].astype(np.int64)     # [S]

    # rope tables (fp32, matching the reference)
    inv_freq = (1.0 / (THETA ** (np.arange(0, ROPE, 2, dtype=np.float32) / ROPE))).astype(np.float32)
    t = pos.astype(np.float32)
    freqs = np.outer(t, inv_freq).astype(np.float32)       # [S, 32]
    emb = np.concatenate([freqs, freqs], -1)               # [S, 64]
    cos = np.cos(emb).astype(np.float32)
    sin = np.sin(emb).astype(np.float32)
    cosT = np.ascontiguousarray(cos.T)                     # [64, S]
    sinT = np.ascontiguousarray(sin.T)
    sinTn = sinT.copy()
    sinTn[:32] = -sinTn[:32]                               # fold rotate_half sign
    cos2 = np.concatenate([cosT, cosT], 0)                 # [128, S]
    sin2n = np.concatenate([sinTn, sinTn], 0)

    perm = np.concatenate([np.arange(0, ROPE, 2), np.arange(1, ROPE, 2)])  # interleave

    # swap-halves permutation matrix (two independent 64 blocks)
    swapp = np.zeros((128, 128), np.float32)
    for m in range(128):
        base = (m // 64) * 64
        i = m % 64
        swapp[base + (i + 32) % 64, m] = 1.0

    maskt = np.triu(np.ones((128, 128), np.float32))

    wq_b = np.asarray(wq_b, np.float32) * np.asarray(q_a_ln_w, np.float32)[None, :]
    kvb = np.asarray(wkv_b, np.float32).reshape(16, NOPE + VD, KVL)
    wkv_a = np.asarray(wkv_a, np.float32)
    wkv_rows = np.concatenate(
        [wkv_a[:KVL], wkv_a[KVL:][perm], wkv_a[KVL:][perm]], 0
    )                                                      # [640, HID]

    hid_T = np.ascontiguousarray(hs.T)                     # [HID, S]
    shared = {
        "hid_t": _bf16(hid_T),
        "wqa_t": _bf16(np.asarray(wq_a, np.float32).T),
        "wkv_t": _bf16(wkv_rows.T),
        "kvln": _tf32_rne(np.asarray(kv_a_ln_w, np.float32)[None, :]),
        "cos2": cos2, "sin2n": sin2n,
        "swapp": _tf32_rne(swapp), "maskt": maskt,
    }

    wo = np.asarray(wo, np.float32)
    in_maps = []
    for core in range(N_CORES):
        h0 = H_PER_CORE * core
        blocks = []
        pe_rows = []
        for h in (h0, h0 + 1):
            blk = wq_b[192 * h : 192 * (h + 1)]
            blocks.append(blk[:NOPE])
            pe_rows.append(blk[NOPE:][perm])
        wqb_re = np.concatenate(blocks + pe_rows, 0)       # [384, QLR]
        wukt = np.stack(
            [np.ascontiguousarray(kvb[h, :NOPE, :].T) for h in (h0, h0 + 1)]
        )                                                  # [2, 512, 128]
        wuv2 = np.concatenate(
            [kvb[h, NOPE:, :].T for h in (h0, h0 + 1)], axis=1
        )                                                  # [512, 256]
        wo_c = np.ascontiguousarray(wo[:, VD * h0 : VD * (h0 + 2)].T)   # [256, HID]
        in_maps.append({
            **shared,
            "hid_own": _bf16(hid_T[:, SHW * core : SHW * (core + 1)]),
            "wqb_t": _bf16(wqb_re.T),
            "wukt": _tf32_rne(wukt),
            "wuv2": _tf32_rne(np.ascontiguousarray(wuv2)),
            "wo_t": _tf32_rne(wo_c),
        })
    return in_maps


def kernel(**inputs):
    from concourse.bass_utils import run_bass_kernel_spmd

    nc = _get_program()
    in_maps = _host_inputs(**inputs)
    res = run_bass_kernel_spmd(nc, in_maps, core_ids=list(range(N_CORES)))
    acc = None
    for r in res.results:
        o = np.asarray(r["out_t"], dtype=np.float32)
        acc = o if acc is None else acc + o
    out = np.ascontiguousarray(acc.T)[None]                # [1, S, HID]
    return out.astype(np.float32)


# revision 22
# speedup vs baseline: 1.0568x; 1.0568x over previous
"""MLA (DeepSeek-style multi-head latent attention) forward on 8 TRN2 NeuronCores.

Sharding: the q_down projection (the largest replicated GEMM) is sharded over
sequence (each core computes its own 256 rows, normalized, bf16) and
AllGathered on-device while the still-replicated ckv projection and the K/V
materialization run on the tensor engine — the collective hides behind ~93us
of PE work. Attention and the output projection are tensor-parallel over
heads (16 heads -> 2 per core); partial wo outputs are summed on host.

Device layout is "feature-major" (features on SBUF partitions, sequence on the
free dim) throughout. Attention uses the prefill-optimal NON-absorbed form:
per-head K (128-dim nope) and V (128-dim) are materialized from the shared
latent once, so scores contract over 192 dims (not 576) and ctx over 128
(not 512). Scores come out k-major ([k, q]); softmax normalization over k is
done with ones-matmuls on the tensor engine.

The projections run with bf16 inputs and weights (half the DMA, same PE rate
as fp32r at free-dim >= 256), accumulating in fp32 PSUM. Attention operands
stay float32r (TF32). wo partials return in bf16 and are summed in fp32 on
host.

Pipeline per core (S=2048; 4 seq-chunks of 512 for full-S phases):
  L:  local q_down shard: q_downT[:, own 256 cols] = wq_a.T^T @ hid_ownT,
      software-pipelined li-groups (sum-of-squares via ACT Square straight
      from PSUM + ones-matmul trail one group behind); rms fold: shard is
      normalized (r_q broadcast via gpsimd) and rounded to bf16; DMA to DRAM
      -> AllGather across the 8 cores.
  C:  ckvT = wkv_a'.T^T @ hidT replicated (RoPE interleave baked into the pe
      rows of wkv_a) -> DRAM spill. Runs while the AllGather flies.
  K:  two-stage pipeline per chunk: latent rms (kv ln folded into the
      broadcast matmul), then per-head k_nopeT (feature-major) and V
      (seq-major, both heads side by side) for the previous chunk; RoPE k_pe.
  B:  post-gather wq_b: qT (all heads' rows for this core) from the gathered
      normalized q_down, PSUM-accumulated over the 12 l-tiles; RoPE q_pe.
  A:  per k-block, software-pipelined over both heads: scoresT -> exp (no
      max subtraction needed: |score*scale| <= ~4) -> causal mask on the
      diagonal block (suffix-sliced matmuls skip fully-masked columns) ->
      ctxT + softmax denominator accumulation in PSUM; per-head epilogues
      deferred into the next chunk's score stream.
  W:  wo partial matmul -> bf16 DRAM outT.
Host: sum the 8 partial outT in fp32, transpose -> [1, S, HID].
"""

import numpy as np

S = 2048
HID = 2048
QLR = 1536
H_PER_CORE = 2
N_CORES = 8
NOPE = 128
ROPE = 64
VD = 128
KVL = 512
EPS = 1e-6
THETA = 10000.0
SCALE = float((NOPE + ROPE) ** -0.5)
NC_ = 4            # seq chunks
CW = 512           # chunk width
SHW = S // N_CORES  # 256-wide local shard
KB = S // 128      # 16 k-blocks
NLT = QLR // 128   # 12 l-tiles


def _tf32_rne(a):
    a = np.ascontiguousarray(a, dtype=np.float32)
    u = a.view(np.uint32).astype(np.uint64)
    u = (u + 0xFFF + ((u >> 13) & 1)) & 0xFFFFE000
    return u.astype(np.uint32).view(np.float32)


def _bf16(a):
    import ml_dtypes
    return np.ascontiguousarray(np.asarray(a, np.float32)).astype(ml_dtypes.bfloat16)


def _build_program():
    import concourse.mybir as mybir
    import concourse.tile as tile
    from concourse import bacc

    f32 = mybir.dt.float32
    f32r = mybir.dt.float32r
    bf16 = mybir.dt.bfloat16
    AF = mybir.ActivationFunctionType
    OP = mybir.AluOpType

    nc = bacc.Bacc("TRN2", target_bir_lowering=False, num_devices=N_CORES)

    hid_own = nc.dram_tensor("hid_own", [HID, SHW], bf16, kind="ExternalInput")
    hid_t = nc.dram_tensor("hid_t", [HID, S], bf16, kind="ExternalInput")
    wqa_t = nc.dram_tensor("wqa_t", [HID, QLR], bf16, kind="ExternalInput")
    wqb_t = nc.dram_tensor("wqb_t", [QLR, 384], bf16, kind="ExternalInput")
    wkv_t = nc.dram_tensor("wkv_t", [HID, 640], bf16, kind="ExternalInput")
    kvln_d = nc.dram_tensor("kvln", [1, KVL], f32r, kind="ExternalInput")
    wukt_d = nc.dram_tensor("wukt", [H_PER_CORE, KVL, NOPE], f32r,
                            kind="ExternalInput")
    wuv2_d = nc.dram_tensor("wuv2", [KVL, H_PER_CORE * VD], f32r,
                            kind="ExternalInput")
    wo_t = nc.dram_tensor("wo_t", [H_PER_CORE * VD, HID], f32r, kind="ExternalInput")
    cos2_d = nc.dram_tensor("cos2", [128, S], f32, kind="ExternalInput")
    sin2n_d = nc.dram_tensor("sin2n", [128, S], f32, kind="ExternalInput")
    swapp_d = nc.dram_tensor("swapp", [128, 128], f32r, kind="ExternalInput")
    maskt_d = nc.dram_tensor("maskt", [128, 128], f32, kind="ExternalInput")
    out_t = nc.dram_tensor("out_t", [HID, S], bf16, kind="ExternalOutput")

    with tile.TileContext(nc) as tc:
        with (
            tc.tile_pool(name="stats", bufs=1) as stats,
            tc.tile_pool(name="dram", bufs=1, space="DRAM") as dram,
        ):
            ones_p = stats.tile([128, 1], f32r)
            nc.vector.memset(ones_p.bitcast(f32), 1.0)
            ones_row = stats.tile([1, 128], f32r)
            nc.vector.memset(ones_row.bitcast(f32), 1.0)
            eps_sb = stats.tile([1, 1], f32)
            nc.vector.memset(eps_sb, EPS)

            ckv_spill = dram.tile([128, 5, S], f32r)
            qdn_shard = dram.tile([QLR, SHW], bf16)
            qdn_full = dram.tile([N_CORES * QLR, SHW], bf16,
                                 addr_space="Shared")

            # ========== Phase L: local q_down shard -> AllGather ==========
            with (
                tc.tile_pool(name="hidop", bufs=1) as hidop,
                tc.tile_pool(name="wqap", bufs=1) as wqap,
                tc.tile_pool(name="lwork", bufs=2) as lwork,
                tc.tile_pool(name="lbig", bufs=1) as lbig,
                tc.tile_pool(name="lpsum", bufs=2, space="PSUM") as lpsum,
                tc.tile_pool(name="lpsum1", bufs=1, space="PSUM") as lpsum1,
            ):
                hid_own_sb = hidop.tile([128, KB, SHW], bf16)
                for kt in range(KB):
                    nc.sync.dma_start(
                        hid_own_sb[:, kt, :],
                        hid_own.ap()[128 * kt : 128 * (kt + 1), :],
                    )
                wqa_sb = wqap.tile([128, KB, QLR], bf16)
                for lg in range(3):
                    nc.sync.dma_start(
                        wqa_sb[:, :, 512 * lg : 512 * (lg + 1)],
                        wqa_t.ap()[:, 512 * lg : 512 * (lg + 1)].rearrange(
                            "(kt p) m -> p kt m", p=128
                        ),
                    )
                qdraw = lbig.tile([128, NLT, SHW], f32)
                qdn_sb = lbig.tile([128, NLT, SHW], bf16)
                ssql = stats.tile([1, SHW], f32)
                rqbl = stats.tile([128, SHW], f32)
                ps_ssq = lpsum1.tile([1, SHW], f32, tag="ssq")

                prev_qd = [None]

                def emit_ssq(li):
                    sq = lwork.tile([128, SHW], f32r, tag="sq")
                    nc.scalar.activation(sq, prev_qd[0], AF.Square)
                    nc.tensor.matmul(
                        ps_ssq, ones_p, sq, start=(li == 0), stop=(li == NLT - 1)
                    )

                for li in range(NLT):
                    ps_qd = lpsum.tile([128, SHW], f32, tag="qd",
                                       name=f"ps_qd{li % 2}")
                    for kt in range(KB):
                        nc.tensor.matmul(
                            ps_qd,
                            wqa_sb[:, kt, 128 * li : 128 * (li + 1)],
                            hid_own_sb[:, kt, :],
                            start=(kt == 0),
                            stop=(kt == KB - 1),
                        )
                    if li > 0:
                        emit_ssq(li - 1)
                    nc.vector.tensor_copy(qdraw[:, li, :], ps_qd)
                    prev_qd[0] = ps_qd
                emit_ssq(NLT - 1)
                nc.scalar.activation(
                    ssql, ps_ssq, AF.Sqrt, scale=1.0 / QLR, bias=eps_sb
                )
                nc.vector.reciprocal_approx_fast(out=ssql, in_=ssql)
                nc.gpsimd.partition_broadcast(rqbl, ssql, channels=128)
                for li in range(NLT):
                    nc.vector.tensor_tensor(
                        qdn_sb[:, li, :], qdraw[:, li, :], rqbl, OP.mult
                    )
                nc.gpsimd.dma_start(
                    qdn_shard.rearrange("(li p) s -> p li s", p=128),
                    qdn_sb,
                )
                nc.gpsimd.collective_compute(
                    "AllGather",
                    mybir.AluOpType.bypass,
                    replica_groups=[list(range(N_CORES))],
                    ins=[qdn_shard.opt()],
                    outs=[qdn_full.opt()],
                )

            # ========== Phase C: replicated ckvT (overlaps the AllGather) =====
            with (
                tc.tile_pool(name="hidp", bufs=1) as hidp,
                tc.tile_pool(name="wkvp", bufs=1) as wkvp,
                tc.tile_pool(name="cwork", bufs=2) as cwork,
                tc.tile_pool(name="cpsum", bufs=2, space="PSUM") as cpsum,
            ):
                hid_sb = hidp.tile([128, KB, S], bf16)
                for kt in range(KB):
                    nc.sync.dma_start(
                        hid_sb[:, kt, :], hid_t.ap()[128 * kt : 128 * (kt + 1), :]
                    )
                wkv_sb = wkvp.tile([128, KB, 640], bf16, tag="wkv")
                for dt in range(5):
                    nc.sync.dma_start(
                        wkv_sb[:, :, 128 * dt : 128 * (dt + 1)],
                        wkv_t.ap()[:, 128 * dt : 128 * (dt + 1)].rearrange(
                            "(kt p) m -> p kt m", p=128
                        ),
                    )
                for c in range(NC_):
                    cs = slice(CW * c, CW * (c + 1))
                    for dt in range(5):
                        ps_ck = cpsum.tile([128, CW], f32, tag="ck")
                        for kt in range(KB):
                            nc.tensor.matmul(
                                ps_ck,
                                wkv_sb[:, kt, 128 * dt : 128 * (dt + 1)],
                                hid_sb[:, kt, cs],
                                start=(kt == 0),
                                stop=(kt == KB - 1),
                            )
                        ckb = cwork.tile([128, CW], f32r, tag="ckb")
                        nc.vector.tensor_copy(ckb, ps_ck)
                        nc.sync.dma_start(ckv_spill[:, dt, cs], ckb)

            # ============ late constants + persistent attention tensors ============
            with (
                tc.tile_pool(name="consts", bufs=1) as consts,
                tc.tile_pool(name="resid", bufs=1) as resid,
            ):
                kvln_sb = consts.tile([1, KVL], f32r)
                nc.sync.dma_start(kvln_sb, kvln_d.ap())
                wukt_sb = consts.tile([128, H_PER_CORE, 4, NOPE], f32r)
                nc.sync.dma_start(
                    wukt_sb, wukt_d.ap().rearrange("h (lt p) n -> p h lt n", p=128)
                )
                wuv2_sb = consts.tile([128, 4, H_PER_CORE * VD], f32r)
                nc.sync.dma_start(
                    wuv2_sb, wuv2_d.ap().rearrange("(lt p) v -> p lt v", p=128)
                )
                cos2_sb = consts.tile([128, S], f32)
                nc.sync.dma_start(cos2_sb, cos2_d.ap())
                sin2n_sb = consts.tile([128, S], f32)
                nc.sync.dma_start(sin2n_sb, sin2n_d.ap())
                swapp_sb = consts.tile([128, 128], f32r)
                nc.sync.dma_start(swapp_sb, swapp_d.ap())
                maskt_sb = consts.tile([128, 128], f32)
                nc.sync.dma_start(maskt_sb, maskt_d.ap())

                kpe = resid.tile([128, S], f32r)          # roped k_peT (2 head copies)
                kn_sb = resid.tile([128, H_PER_CORE, S], f32r)  # per-head k_nopeT
                v_sb = resid.tile([128, KB, H_PER_CORE * VD], f32r)  # V seq-major
                ctxa = resid.tile([128, H_PER_CORE, S], f32r)
                wo_sb = resid.tile([128, H_PER_CORE, HID], f32r)
                nc.sync.dma_start(
                    wo_sb, wo_t.ap().rearrange("(h p) m -> p h m", p=128)
                )
                qtr = resid.tile([128, 3, S], f32r)       # post-gather q (r_q folded)
                qspe = resid.tile([128, S], f32r)         # roped q_peT

                # ===== Phase K: latent rms + per-head K/V materialization =====
                # Two-stage pipeline: rms chain of chunk c overlaps the
                # materialization matmuls of chunk c-1 on the PE.
                with (
                    tc.tile_pool(name="kwork", bufs=2) as kwork,
                    tc.tile_pool(name="kpsum", bufs=2, space="PSUM") as kpsum,
                    tc.tile_pool(name="kpsum1", bufs=1, space="PSUM") as kpsum1,
                ):
                    ksn_tiles = [None] * NC_
                    ck_tiles = [None] * NC_
                    rk_tiles = [None] * NC_

                    def emit_rms_a(c):
                        cs = slice(CW * c, CW * (c + 1))
                        ck = kwork.tile([128, 5, CW], f32r, tag="ck",
                                        name=f"ck{c}")
                        nc.sync.dma_start(ck, ckv_spill[:, :, cs])
                        ps_ssqk = kpsum1.tile([1, CW], f32, tag="ssqk")
                        for j in range(4):
                            sqk = kwork.tile([128, CW], f32r, tag="sqk")
                            nc.scalar.activation(
                                sqk, ck[:, j, :].bitcast(f32), AF.Square
                            )
                            nc.tensor.matmul(
                                ps_ssqk, ones_p, sqk, start=(j == 0), stop=(j == 3)
                            )
                        rk = kwork.tile([1, CW], f32, tag="rk")
                        nc.scalar.activation(
                            rk, ps_ssqk, AF.Sqrt, scale=1.0 / KVL, bias=eps_sb
                        )
                        nc.vector.reciprocal_approx_fast(out=rk, in_=rk)
                        rk_r = kwork.tile([1, CW], f32r, tag="rkr")
                        nc.vector.tensor_copy(rk_r, rk)
                        ck_tiles[c] = ck
                        rk_tiles[c] = rk_r

                    def emit_rms_b(c):
                        ck = ck_tiles[c]
                        rk_r = rk_tiles[c]
                        ksn_c = kwork.tile([128, 4, CW], f32r, tag="ksn",
                                           name=f"ksn{c}")
                        for j in range(4):
                            ps_b = kpsum1.tile([128, CW], f32, tag="bc")
                            nc.tensor.matmul(
                                ps_b,
                                kvln_sb[0:1, 128 * j : 128 * (j + 1)],
                                rk_r,
                                start=True,
                                stop=True,
                            )
                            nc.vector.tensor_tensor(
                                ksn_c[:, j, :], ck[:, j, :].bitcast(f32), ps_b,
                                OP.mult
                            )
                        ksn_tiles[c] = ksn_c

                    def emit_mat(c):
                        cs = slice(CW * c, CW * (c + 1))
                        ksn_c = ksn_tiles[c]
                        ck = ck_tiles[c]
                        for h in range(H_PER_CORE):
                            ps_k = kpsum.tile([128, CW], f32, tag="kn")
                            for lt in range(4):
                                nc.tensor.matmul(
                                    ps_k,
                                    wukt_sb[:, h, lt, :],
                                    ksn_c[:, lt, :],
                                    start=(lt == 0),
                                    stop=(lt == 3),
                                )
                            nc.vector.tensor_copy(kn_sb[:, h, cs], ps_k)
                        for b in range(4):
                            ps_v = kpsum.tile([128, H_PER_CORE * VD], f32, tag="v")
                            for lt in range(4):
                                nc.tensor.matmul(
                                    ps_v,
                                    ksn_c[:, lt, 128 * b : 128 * (b + 1)],
                                    wuv2_sb[:, lt, :],
                                    start=(lt == 0),
                                    stop=(lt == 3),
                                )
                            nc.vector.tensor_copy(v_sb[:, 4 * c + b, :], ps_v)
                        # k_pe rope (both 64-row copies at once)
                        ps_sw = kpsum1.tile([128, CW], f32, tag="sw")
                        nc.tensor.matmul(
                            ps_sw, swapp_sb, ck[:, 4, :], start=True, stop=True
                        )
                        t1 = kwork.tile([128, CW], f32, tag="t1")
                        nc.vector.tensor_tensor(
                            t1, ck[:, 4, :].bitcast(f32), cos2_sb[:, cs], OP.mult
                        )
                        t2 = kwork.tile([128, CW], f32, tag="t2")
                        nc.vector.tensor_tensor(t2, ps_sw, sin2n_sb[:, cs], OP.mult)
                        nc.vector.tensor_tensor(kpe[:, cs], t1, t2, OP.add)

                    emit_rms_a(0)
                    emit_rms_b(0)
                    for c in range(1, NC_):
                        emit_rms_a(c)
                        emit_mat(c - 1)
                        emit_rms_b(c)
                    emit_mat(NC_ - 1)

                # ===== Phase B: post-gather wq_b + q rope =====
                with (
                    tc.tile_pool(name="bqp", bufs=1) as bqp,
                    tc.tile_pool(name="bwork", bufs=2) as bwork,
                    tc.tile_pool(name="bpsum", bufs=2, space="PSUM") as bpsum,
                    tc.tile_pool(name="bmisc", bufs=1, space="PSUM") as bmisc,
                ):
                    qdn_all = bqp.tile([128, NLT, N_CORES, SHW], bf16)
                    qdn_src = qdn_full.rearrange(
                        "(r li p) s -> p li r s", p=128, li=NLT
                    )
                    for li in range(NLT):
                        nc.gpsimd.dma_start(
                            qdn_all[:, li, :, :], qdn_src[:, li, :, :]
                        )
                    wqb_sb = bqp.tile([128, NLT, 384], bf16)
                    nc.sync.dma_start(
                        wqb_sb,
                        wqb_t.ap().rearrange("(li p) m -> p li m", p=128),
                    )
                    for c in range(NC_):
                        cs = slice(CW * c, CW * (c + 1))
                        for dt in range(3):
                            ps_qt = bpsum.tile([128, CW], f32, tag="qt")
                            for li in range(NLT):
                                nc.tensor.matmul(
                                    ps_qt,
                                    wqb_sb[:, li, 128 * dt : 128 * (dt + 1)],
                                    qdn_all[:, li, 2 * c : 2 * c + 2, :],
                                    start=(li == 0),
                                    stop=(li == NLT - 1),
                                )
                            nc.vector.tensor_copy(qtr[:, dt, cs], ps_qt)
                        # rope q_pe (both heads stacked)
                        ps_sw = bmisc.tile([128, CW], f32, tag="misc",
                                           name="ps_swq")
                        nc.tensor.matmul(
                            ps_sw, swapp_sb, qtr[:, 2, cs], start=True, stop=True
                        )
                        t1 = bwork.tile([128, CW], f32, tag="t1")
                        nc.vector.tensor_tensor(
                            t1, qtr[:, 2, cs].bitcast(f32), cos2_sb[:, cs], OP.mult
                        )
                        t2 = bwork.tile([128, CW], f32, tag="t2")
                        nc.vector.tensor_tensor(t2, ps_sw, sin2n_sb[:, cs], OP.mult)
                        nc.vector.tensor_tensor(qspe[:, cs], t1, t2, OP.add)

                # ================= Phase A: attention =================
                with (
                    tc.tile_pool(name="asm", bufs=2) as asm,
                    tc.tile_pool(name="attp", bufs=3) as attp,
                    tc.tile_pool(name="aacc", bufs=1, space="PSUM") as aacc,
                    tc.tile_pool(name="ascore", bufs=2, space="PSUM") as ascore,
                    tc.tile_pool(name="ascr1", bufs=1, space="PSUM") as ascr1,
                    tc.tile_pool(name="amisc", bufs=1, space="PSUM") as amisc,
                ):
                    pending_epilogue = [None]

                    for c in range(NC_):
                        cs = slice(CW * c, CW * (c + 1))
                        nj = 4 * c + 4
                        ps_ctx = [
                            aacc.tile([128, CW], f32, tag=f"ctx{h}",
                                      name=f"ps_ctx{h}")
                            for h in range(H_PER_CORE)
                        ]
                        ps_sum = [
                            aacc.tile([1, CW], f32, tag=f"sum{h}",
                                      name=f"ps_sum{h}")
                            for h in range(H_PER_CORE)
                        ]
                        # software pipeline: scores(j) on PE, then ctx(j-1);
                        # exp(j) on ACT overlaps ctx(j-1)+scores(j+1) on PE.
                        atts = [[None, None] for _ in range(nj)]
                        offs = [0 if j < 4 * c else 128 * (j - 4 * c)
                                for j in range(nj)]

                        def emit_scores(j, c=c, nj=nj, offs=offs, atts=atts):
                            off = offs[j]
                            q0 = CW * c + off
                            qs = slice(q0, CW * (c + 1))
                            for h in range(H_PER_CORE):
                                pool = ascore if h == 0 else ascr1
                                ps_s = pool.tile([128, CW], f32, tag=f"sc{h}",
                                                 name=f"ps_s{h}")
                                nc.tensor.matmul(
                                    ps_s[:, off:],
                                    kn_sb[:, h, 128 * j : 128 * (j + 1)],
                                    qtr[:, h, qs],
                                    start=True,
                                    stop=False,
                                )
                                nc.tensor.matmul(
                                    ps_s[:, off:],
                                    kpe[64 * h : 64 * (h + 1),
                                        128 * j : 128 * (j + 1)],
                                    qspe[64 * h : 64 * (h + 1), qs],
                                    start=False,
                                    stop=True,
                                )
                                att = attp.tile([128, CW], f32r, tag=f"att{h}",
                                                name=f"att{h}")
                                nc.scalar.activation(
                                    att[:, off:], ps_s[:, off:], AF.Exp, scale=SCALE
                                )
                                if j >= 4 * c:
                                    nc.vector.tensor_tensor(
                                        att[:, off : off + 128],
                                        att[:, off : off + 128].bitcast(f32),
                                        maskt_sb,
                                        OP.mult,
                                    )
                                atts[j][h] = att

                        def emit_ctx(j, c=c, nj=nj, offs=offs, atts=atts,
                                     ps_ctx=ps_ctx, ps_sum=ps_sum):
                            off = offs[j]
                            for h in range(H_PER_CORE):
                                nc.tensor.matmul(
                                    ps_ctx[h][:, off:],
                                    v_sb[:, j, VD * h : VD * (h + 1)],
                                    atts[j][h][:, off:],
                                    start=(j == 0),
                                    stop=(j == nj - 1),
                                )
                                nc.tensor.matmul(
                                    ps_sum[h][:, off:],
                                    ones_p,
                                    atts[j][h][:, off:],
                                    start=(j == 0),
                                    stop=(j == nj - 1),
                                )

                        emit_scores(0)
                        emit_scores(1)
                        if pending_epilogue[0] is not None:
                            pending_epilogue[0]()
                        emit_ctx(0)
                        for j in range(2, nj):
                            emit_scores(j)
                            emit_ctx(j - 1)
                        emit_ctx(nj - 1)

                        def epilogue(c=c, cs=cs, ps_ctx=ps_ctx, ps_sum=ps_sum):
                            for h in range(H_PER_CORE):
                                s_sb = asm.tile([1, CW], f32, tag="ssb")
                                nc.vector.tensor_copy(s_sb, ps_sum[h])
                                nc.vector.reciprocal_approx_fast(
                                    out=s_sb, in_=s_sb
                                )
                                rs_r = asm.tile([1, CW], f32r, tag="rsr")
                                nc.vector.tensor_copy(rs_r, s_sb)
                                ps_rb2 = amisc.tile([128, CW], f32, tag="misc",
                                                    name="ps_rb2")
                                nc.tensor.matmul(
                                    ps_rb2, ones_row, rs_r, start=True, stop=True
                                )
                                rsb = asm.tile([128, CW], f32, tag="rsb")
                                nc.vector.tensor_copy(rsb, ps_rb2)
                                nc.vector.tensor_tensor(
                                    ctxa[:, h, cs], ps_ctx[h], rsb, OP.mult
                                )

                        pending_epilogue[0] = epilogue
                    pending_epilogue[0]()

                # ================= Phase W: output projection =================
                with (
                    tc.tile_pool(name="obp", bufs=3) as obp,
                    tc.tile_pool(name="wpsum", bufs=2, space="PSUM") as wpsum,
                ):
                    for c in range(NC_):
                        cs = slice(CW * c, CW * (c + 1))
                        for ht in range(KB):
                            ps_o = wpsum.tile([128, CW], f32, tag="o")
                            for h in range(H_PER_CORE):
                                nc.tensor.matmul(
                                    ps_o,
                                    wo_sb[:, h, 128 * ht : 128 * (ht + 1)],
                                    ctxa[:, h, cs],
                                    start=(h == 0),
                                    stop=(h == H_PER_CORE - 1),
                                )
                            ob = obp.tile([128, CW], bf16, tag="ob")
                            nc.vector.tensor_copy(ob, ps_o)
                            nc.sync.dma_start(
                                out_t.ap()[128 * ht : 128 * (ht + 1), cs], ob
                            )

    nc.finalize()
    return nc


_PROGRAM = None


def _get_program():
    global _PROGRAM
    if _PROGRAM is None:
        _PROGRAM = _build_program()
    return _PROGRAM


def _host_inputs(hidden_states, position_ids, wq_a, q_a_ln_w, wq_b, wkv_a,
                 kv_a_ln_w, wkv_b, wo):
    """Build the 8 per-core input maps."""
    hs = np.asarray(hidden_states, np.float32)[0]          # [S, HID]
    pos = np.asarray(position_ids)[0].astype(np.int64)     # [S]

    # rope tables (fp32, matching the reference)
    inv_freq = (1.0 / (THETA ** (np.arange(0, ROPE, 2, dtype=np.float32) / ROPE))).astype(np.float32)
    t = pos.astype(np.float32)
    freqs = np.outer(t, inv_freq).astype(np.float32)       # [S, 32]
    emb = np.concatenate([freqs, freqs], -1)               # [S, 64]
    cos = np.cos(emb).astype(np.float32)
    sin = np.sin(emb).astype(np.float32)
    cosT = np.ascontiguousarray(cos.T)                     # [64, S]
    sinT = np.ascontiguousarray(sin.T)
    sinTn = sinT.copy()
    sinTn[:32] = -sinTn[:32]                               # fold rotate_half sign
    cos2 = np.concatenate([cosT, cosT], 0)                 # [128, S]
    sin2n = np.concatenate([sinTn, sinTn], 0)

    perm = np.concatenate([np.arange(0, ROPE, 2), np.arange(1, ROPE, 2)])  # interleave

    # swap-halves permutation matrix (two independent 64 blocks)
    swapp = np.zeros((128, 128), np.float32)
    for m in range(128):
        base = (m // 64) * 64
        i = m % 64
        swapp[base + (i + 32) % 64, m] = 1.0

    maskt = np.triu(np.ones((128, 128), np.float32))

    wq_b = np.asarray(wq_b, np.float32) * np.asarray(q_a_ln_w, np.float32)[None, :]
    kvb = np.asarray(wkv_b, np.float32).reshape(16, NOPE + VD, KVL)
    wkv_a = np.asarray(wkv_a, np.float32)
    wkv_rows = np.concatenate(
        [wkv_a[:KVL], wkv_a[KVL:][perm], wkv_a[KVL:][perm]], 0
    )                                                      # [640, HID]

    hid_T = np.ascontiguousarray(hs.T)                     # [HID, S]
    shared = {
        "hid_t": _bf16(hid_T),
        "wqa_t": _bf16(np.asarray(wq_a, np.float32).T),
        "wkv_t": _bf16(wkv_rows.T),
        "kvln": _tf32_rne(np.asarray(kv_a_ln_w, np.float32)[None, :]),
        "cos2": cos2, "sin2n": sin2n,
        "swapp": _tf32_rne(swapp), "maskt": maskt,
    }

    wo = np.asarray(wo, np.float32)
    in_maps = []
    for core in range(N_CORES):
        h0 = H_PER_CORE * core
        blocks = []
        pe_rows = []
        for h in (h0, h0 + 1):
            blk = wq_b[192 * h : 192 * (h + 1)]
            blocks.append(blk[:NOPE])
            pe_rows.append(blk[NOPE:][perm])
        wqb_re = np.concatenate(blocks + pe_rows, 0)       # [384, QLR]
        wukt = np.stack(
            [np.ascontiguousarray(kvb[h, :NOPE, :].T) for h in (h0, h0 + 1)]
        )                                                  # [2, 512, 128]
        wuv2 = np.concatenate(
            [kvb[h, NOPE:, :].T for h in (h0, h0 + 1)], axis=1
        )                                                  # [512, 256]
        wo_c = np.ascontiguousarray(wo[:, VD * h0 : VD * (h0 + 2)].T)   # [256, HID]
        in_maps.append({
            **shared,
            "hid_own": _bf16(hid_T[:, SHW * core : SHW * (core + 1)]),
            "wqb_t": _bf16(wqb_re.T),
            "wukt": _tf32_rne(wukt),
            "wuv2": _tf32_rne(np.ascontiguousarray(wuv2)),
            "wo_t": _tf32_rne(wo_c),
        })
    return in_maps


def kernel(**inputs):
    from concourse.bass_utils import run_bass_kernel_spmd

    nc = _get_program()
    in_maps = _host_inputs(**inputs)
    res = run_bass_kernel_spmd(nc, in_maps, core_ids=list(range(N_CORES)))
    acc = None
    for r in res.results:
        o = np.asarray(r["out_t"], dtype=np.float32)
        acc = o if acc is None else acc + o
    out = np.ascontiguousarray(acc.T)[None]                # [1, S, HID]
    return out.astype(np.float32)


# revision 27
# speedup vs baseline: 1.1156x; 1.0556x over previous
"""MLA (DeepSeek-style multi-head latent attention) forward on 8 TRN2 NeuronCores.

Sharding: the q_down projection (the largest replicated GEMM) is sharded over
sequence (each core computes its own 256 rows, normalized, bf16) and
AllGathered on-device while the still-replicated ckv projection and the K/V
materialization run on the tensor engine — the collective hides behind ~93us
of PE work. Attention and the output projection are tensor-parallel over
heads (16 heads -> 2 per core); partial wo outputs are summed on host.

Device layout is "feature-major" (features on SBUF partitions, sequence on the
free dim) throughout. Attention uses the prefill-optimal NON-absorbed form:
per-head K (128-dim nope) and V (128-dim) are materialized from the shared
latent once, so scores contract over 192 dims (not 576) and ctx over 128
(not 512). Scores come out k-major ([k, q]); softmax normalization over k is
done with ones-matmuls on the tensor engine.

The projections run with bf16 inputs and weights (half the DMA, same PE rate
as fp32r at free-dim >= 256), accumulating in fp32 PSUM. Attention operands
stay float32r (TF32). wo partials return in bf16 and are summed in fp32 on
host.

Pipeline per core (S=2048; 4 seq-chunks of 512 for full-S phases):
  L:  local q_down shard: q_downT[:, own 256 cols] = wq_a.T^T @ hid_ownT,
      software-pipelined li-groups (sum-of-squares via ACT Square straight
      from PSUM + ones-matmul trail one group behind); rms fold: shard is
      normalized (r_q broadcast via gpsimd) and rounded to bf16; DMA to DRAM
      -> AllGather across the 8 cores.
  C:  ckvT = wkv_a'.T^T @ hidT replicated (RoPE interleave baked into the pe
      rows of wkv_a) -> DRAM spill. Runs while the AllGather flies.
  K:  two-stage pipeline per chunk: latent rms (kv ln folded into the
      broadcast matmul), then per-head k_nopeT (feature-major) and V
      (seq-major, both heads side by side) for the previous chunk; RoPE k_pe.
  B:  post-gather wq_b: qT (all heads' rows for this core) from the gathered
      normalized q_down, PSUM-accumulated over the 12 l-tiles; RoPE q_pe.
  A:  per k-block, software-pipelined over both heads: scoresT -> exp (no
      max subtraction needed: |score*scale| <= ~4) -> causal mask on the
      diagonal block (suffix-sliced matmuls skip fully-masked columns) ->
      ctxT + softmax denominator accumulation in PSUM; per-head epilogues
      deferred into the next chunk's score stream.
  W:  wo partial matmul -> bf16 DRAM outT.
Host: sum the 8 partial outT in fp32, transpose -> [1, S, HID].
"""

import numpy as np

S = 2048
HID = 2048
QLR = 1536
H_PER_CORE = 2
N_CORES = 8
NOPE = 128
ROPE = 64
VD = 128
KVL = 512
EPS = 1e-6
THETA = 10000.0
SCALE = float((NOPE + ROPE) ** -0.5)
NC_ = 4            # seq chunks
CW = 512           # chunk width
SHW = S // N_CORES  # 256-wide local shard
KB = S // 128      # 16 k-blocks
NLT = QLR // 128   # 12 l-tiles


def _tf32_rne(a):
    a = np.ascontiguousarray(a, dtype=np.float32)
    u = a.view(np.uint32).astype(np.uint64)
    u = (u + 0xFFF + ((u >> 13) & 1)) & 0xFFFFE000
    return u.astype(np.uint32).view(np.float32)


def _bf16(a):
    import ml_dtypes
    return np.ascontiguousarray(np.asarray(a, np.float32)).astype(ml_dtypes.bfloat16)


def _build_program():
    import concourse.mybir as mybir
    import concourse.tile as tile
    from concourse import bacc

    f32 = mybir.dt.float32
    f32r = mybir.dt.float32r
    bf16 = mybir.dt.bfloat16
    AF = mybir.ActivationFunctionType
    OP = mybir.AluOpType

    nc = bacc.Bacc("TRN2", target_bir_lowering=False, num_devices=N_CORES)

    hid_own = nc.dram_tensor("hid_own", [HID, SHW], bf16, kind="ExternalInput")
    hid_t = nc.dram_tensor("hid_t", [HID, S], bf16, kind="ExternalInput")
    wqa_t = nc.dram_tensor("wqa_t", [HID, QLR], bf16, kind="ExternalInput")
    wqb_t = nc.dram_tensor("wqb_t", [QLR, 384], bf16, kind="ExternalInput")
    wkv_t = nc.dram_tensor("wkv_t", [HID, 640], bf16, kind="ExternalInput")
    kvln_d = nc.dram_tensor("kvln", [1, KVL], f32r, kind="ExternalInput")
    wukt_d = nc.dram_tensor("wukt", [H_PER_CORE, KVL, NOPE], f32r,
                            kind="ExternalInput")
    wuv2_d = nc.dram_tensor("wuv2", [KVL, H_PER_CORE * VD], f32r,
                            kind="ExternalInput")
    wo_t = nc.dram_tensor("wo_t", [H_PER_CORE * VD, HID], f32r, kind="ExternalInput")
    cos2_d = nc.dram_tensor("cos2", [128, S], f32, kind="ExternalInput")
    sin2n_d = nc.dram_tensor("sin2n", [128, S], f32, kind="ExternalInput")
    swapp_d = nc.dram_tensor("swapp", [128, 128], f32r, kind="ExternalInput")
    maskt_d = nc.dram_tensor("maskt", [128, 128], f32, kind="ExternalInput")
    out_t = nc.dram_tensor("out_t", [HID, S], bf16, kind="ExternalOutput")

    with tile.TileContext(nc) as tc:
        with (
            tc.tile_pool(name="stats", bufs=1) as stats,
            tc.tile_pool(name="dram", bufs=1, space="DRAM") as dram,
        ):
            ones_p = stats.tile([128, 1], f32r)
            nc.vector.memset(ones_p.bitcast(f32), 1.0)
            ones_row = stats.tile([1, 128], f32r)
            nc.vector.memset(ones_row.bitcast(f32), 1.0)
            eps_sb = stats.tile([1, 1], f32)
            nc.vector.memset(eps_sb, EPS)

            ckv_spill = dram.tile([128, 5, S], f32r)
            qdn_shard = dram.tile([QLR, SHW], bf16)
            qdn_full = dram.tile([N_CORES * QLR, SHW], bf16,
                                 addr_space="Shared")

            # ========== Phase L: local q_down shard -> AllGather ==========
            # hid / wkv for phase C are DMA'd up front: once the AllGather
            # launches it monopolizes the DMA engines, so everything phase C
            # needs must already be on-chip.
            hidp_ctx = tc.tile_pool(name="hidp", bufs=1)
            hidp = hidp_ctx.__enter__()
            wkvp_ctx = tc.tile_pool(name="wkvp", bufs=1)
            wkvp = wkvp_ctx.__enter__()
            with (
                tc.tile_pool(name="hidop", bufs=1) as hidop,
                tc.tile_pool(name="wqap", bufs=1) as wqap,
                tc.tile_pool(name="lwork", bufs=2) as lwork,
                tc.tile_pool(name="lbig", bufs=1) as lbig,
                tc.tile_pool(name="lpsum", bufs=2, space="PSUM") as lpsum,
                tc.tile_pool(name="lpsum1", bufs=1, space="PSUM") as lpsum1,
            ):
                hid_own_sb = hidop.tile([128, KB, SHW], bf16)
                for kt in range(KB):
                    nc.sync.dma_start(
                        hid_own_sb[:, kt, :],
                        hid_own.ap()[128 * kt : 128 * (kt + 1), :],
                    )
                wqa_sb = wqap.tile([128, KB, QLR], bf16)
                for lg in range(3):
                    nc.sync.dma_start(
                        wqa_sb[:, :, 512 * lg : 512 * (lg + 1)],
                        wqa_t.ap()[:, 512 * lg : 512 * (lg + 1)].rearrange(
                            "(kt p) m -> p kt m", p=128
                        ),
                    )
                hid_sb = hidp.tile([128, KB, S], bf16)
                for kt in range(KB):
                    nc.sync.dma_start(
                        hid_sb[:, kt, :], hid_t.ap()[128 * kt : 128 * (kt + 1), :]
                    )
                wkv_sb = wkvp.tile([128, KB, 640], bf16, tag="wkv")
                for dt in range(5):
                    nc.sync.dma_start(
                        wkv_sb[:, :, 128 * dt : 128 * (dt + 1)],
                        wkv_t.ap()[:, 128 * dt : 128 * (dt + 1)].rearrange(
                            "(kt p) m -> p kt m", p=128
                        ),
                    )
                qdraw = lbig.tile([128, NLT, SHW], f32)
                qdn_sb = lbig.tile([128, NLT, SHW], bf16)
                ssql = stats.tile([1, SHW], f32)
                rqbl = stats.tile([128, SHW], f32)
                ps_ssq = lpsum1.tile([1, SHW], f32, tag="ssq")

                prev_qd = [None]

                def emit_ssq(li):
                    sq = lwork.tile([128, SHW], f32r, tag="sq")
                    nc.scalar.activation(sq, prev_qd[0], AF.Square)
                    nc.tensor.matmul(
                        ps_ssq, ones_p, sq, start=(li == 0), stop=(li == NLT - 1)
                    )

                for li in range(NLT):
                    ps_qd = lpsum.tile([128, SHW], f32, tag="qd",
                                       name=f"ps_qd{li % 2}")
                    for kt in range(KB):
                        nc.tensor.matmul(
                            ps_qd,
                            wqa_sb[:, kt, 128 * li : 128 * (li + 1)],
                            hid_own_sb[:, kt, :],
                            start=(kt == 0),
                            stop=(kt == KB - 1),
                        )
                    if li > 0:
                        emit_ssq(li - 1)
                    nc.vector.tensor_copy(qdraw[:, li, :], ps_qd)
                    prev_qd[0] = ps_qd
                emit_ssq(NLT - 1)
                nc.scalar.activation(
                    ssql, ps_ssq, AF.Sqrt, scale=1.0 / QLR, bias=eps_sb
                )
                nc.vector.reciprocal_approx_fast(out=ssql, in_=ssql)
                nc.gpsimd.partition_broadcast(rqbl, ssql, channels=128)
                for li in range(NLT):
                    nc.vector.tensor_tensor(
                        qdn_sb[:, li, :], qdraw[:, li, :], rqbl, OP.mult
                    )
                nc.gpsimd.dma_start(
                    qdn_shard.rearrange("(li p) s -> p li s", p=128),
                    qdn_sb,
                )
                nc.gpsimd.collective_compute(
                    "AllGather",
                    mybir.AluOpType.bypass,
                    replica_groups=[list(range(N_CORES))],
                    ins=[qdn_shard.opt()],
                    outs=[qdn_full.opt()],
                )

            # ========== Phase C: replicated ckvT (overlaps the AllGather) =====
            with (
                tc.tile_pool(name="cwork", bufs=2) as cwork,
                tc.tile_pool(name="cpsum", bufs=2, space="PSUM") as cpsum,
            ):
                for c in range(NC_):
                    cs = slice(CW * c, CW * (c + 1))
                    for dt in range(5):
                        ps_ck = cpsum.tile([128, CW], f32, tag="ck")
                        for kt in range(KB):
                            nc.tensor.matmul(
                                ps_ck,
                                wkv_sb[:, kt, 128 * dt : 128 * (dt + 1)],
                                hid_sb[:, kt, cs],
                                start=(kt == 0),
                                stop=(kt == KB - 1),
                            )
                        ckb = cwork.tile([128, CW], f32r, tag="ckb")
                        nc.vector.tensor_copy(ckb, ps_ck)
                        nc.sync.dma_start(ckv_spill[:, dt, cs], ckb)
            wkvp_ctx.__exit__(None, None, None)
            hidp_ctx.__exit__(None, None, None)

            # ============ late constants + persistent attention tensors ============
            with (
                tc.tile_pool(name="consts", bufs=1) as consts,
                tc.tile_pool(name="resid", bufs=1) as resid,
            ):
                kvln_sb = consts.tile([1, KVL], f32r)
                nc.sync.dma_start(kvln_sb, kvln_d.ap())
                wukt_sb = consts.tile([128, H_PER_CORE, 4, NOPE], f32r)
                nc.sync.dma_start(
                    wukt_sb, wukt_d.ap().rearrange("h (lt p) n -> p h lt n", p=128)
                )
                wuv2_sb = consts.tile([128, 4, H_PER_CORE * VD], f32r)
                nc.sync.dma_start(
                    wuv2_sb, wuv2_d.ap().rearrange("(lt p) v -> p lt v", p=128)
                )
                cos2_sb = consts.tile([128, S], f32)
                nc.sync.dma_start(cos2_sb, cos2_d.ap())
                sin2n_sb = consts.tile([128, S], f32)
                nc.sync.dma_start(sin2n_sb, sin2n_d.ap())
                swapp_sb = consts.tile([128, 128], f32r)
                nc.sync.dma_start(swapp_sb, swapp_d.ap())
                maskt_sb = consts.tile([128, 128], f32)
                nc.sync.dma_start(maskt_sb, maskt_d.ap())

                kpe = resid.tile([128, S], f32r)          # roped k_peT (2 head copies)
                kn_sb = resid.tile([128, H_PER_CORE, S], f32r)  # per-head k_nopeT
                v_sb = resid.tile([128, KB, H_PER_CORE * VD], f32r)  # V seq-major
                ctxa = resid.tile([128, H_PER_CORE, S], f32r)
                wo_sb = resid.tile([128, H_PER_CORE, HID], f32r)
                nc.sync.dma_start(
                    wo_sb, wo_t.ap().rearrange("(h p) m -> p h m", p=128)
                )
                qtr = resid.tile([128, 3, S], f32r)       # post-gather q (r_q folded)
                qspe = resid.tile([128, S], f32r)         # roped q_peT
                wqb_sb = resid.tile([128, NLT, 384], bf16)
                nc.sync.dma_start(
                    wqb_sb,
                    wqb_t.ap().rearrange("(li p) m -> p li m", p=128),
                )

                # ===== Phase K: latent rms + per-head K/V materialization =====
                # Two-stage pipeline: rms chain of chunk c overlaps the
                # materialization matmuls of chunk c-1 on the PE.
                with (
                    tc.tile_pool(name="kwork", bufs=2) as kwork,
                    tc.tile_pool(name="kpsum", bufs=2, space="PSUM") as kpsum,
                    tc.tile_pool(name="kpsum1", bufs=1, space="PSUM") as kpsum1,
                ):
                    ksn_tiles = [None] * NC_
                    ck_tiles = [None] * NC_
                    rk_tiles = [None] * NC_

                    def emit_rms_a(c):
                        cs = slice(CW * c, CW * (c + 1))
                        ck = kwork.tile([128, 5, CW], f32r, tag="ck",
                                        name=f"ck{c}")
                        nc.sync.dma_start(ck, ckv_spill[:, :, cs])
                        ps_ssqk = kpsum1.tile([1, CW], f32, tag="ssqk")
                        for j in range(4):
                            sqk = kwork.tile([128, CW], f32r, tag="sqk")
                            nc.scalar.activation(
                                sqk, ck[:, j, :].bitcast(f32), AF.Square
                            )
                            nc.tensor.matmul(
                                ps_ssqk, ones_p, sqk, start=(j == 0), stop=(j == 3)
                            )
                        rk = kwork.tile([1, CW], f32, tag="rk")
                        nc.scalar.activation(
                            rk, ps_ssqk, AF.Sqrt, scale=1.0 / KVL, bias=eps_sb
                        )
                        nc.vector.reciprocal_approx_fast(out=rk, in_=rk)
                        rk_r = kwork.tile([1, CW], f32r, tag="rkr")
                        nc.vector.tensor_copy(rk_r, rk)
                        ck_tiles[c] = ck
                        rk_tiles[c] = rk_r

                    def emit_rms_b(c):
                        ck = ck_tiles[c]
                        rk_r = rk_tiles[c]
                        ksn_c = kwork.tile([128, 4, CW], f32r, tag="ksn",
                                           name=f"ksn{c}")
                        for j in range(4):
                            ps_b = kpsum1.tile([128, CW], f32, tag="bc")
                            nc.tensor.matmul(
                                ps_b,
                                kvln_sb[0:1, 128 * j : 128 * (j + 1)],
                                rk_r,
                                start=True,
                                stop=True,
                            )
                            nc.vector.tensor_tensor(
                                ksn_c[:, j, :], ck[:, j, :].bitcast(f32), ps_b,
                                OP.mult
                            )
                        ksn_tiles[c] = ksn_c

                    def emit_mat(c):
                        cs = slice(CW * c, CW * (c + 1))
                        ksn_c = ksn_tiles[c]
                        ck = ck_tiles[c]
                        for h in range(H_PER_CORE):
                            ps_k = kpsum.tile([128, CW], f32, tag="kn")
                            for lt in range(4):
                                nc.tensor.matmul(
                                    ps_k,
                                    wukt_sb[:, h, lt, :],
                                    ksn_c[:, lt, :],
                                    start=(lt == 0),
                                    stop=(lt == 3),
                                )
                            nc.vector.tensor_copy(kn_sb[:, h, cs], ps_k)
                        for b in range(4):
                            ps_v = kpsum.tile([128, H_PER_CORE * VD], f32, tag="v")
                            for lt in range(4):
                                nc.tensor.matmul(
                                    ps_v,
                                    ksn_c[:, lt, 128 * b : 128 * (b + 1)],
                                    wuv2_sb[:, lt, :],
                                    start=(lt == 0),
                                    stop=(lt == 3),
                                )
                            nc.vector.tensor_copy(v_sb[:, 4 * c + b, :], ps_v)
                        # k_pe rope (both 64-row copies at once)
                        ps_sw = kpsum1.tile([128, CW], f32, tag="sw")
                        nc.tensor.matmul(
                            ps_sw, swapp_sb, ck[:, 4, :], start=True, stop=True
                        )
                        t1 = kwork.tile([128, CW], f32, tag="t1")
                        nc.vector.tensor_tensor(
                            t1, ck[:, 4, :].bitcast(f32), cos2_sb[:, cs], OP.mult
                        )
                        t2 = kwork.tile([128, CW], f32, tag="t2")
                        nc.vector.tensor_tensor(t2, ps_sw, sin2n_sb[:, cs], OP.mult)
                        nc.vector.tensor_tensor(kpe[:, cs], t1, t2, OP.add)

                    emit_rms_a(0)
                    emit_rms_b(0)
                    for c in range(1, NC_):
                        emit_rms_a(c)
                        emit_mat(c - 1)
                        emit_rms_b(c)
                    emit_mat(NC_ - 1)

                # ===== Phase B: post-gather wq_b + q rope =====
                with (
                    tc.tile_pool(name="bqp", bufs=1) as bqp,
                    tc.tile_pool(name="bwork", bufs=2) as bwork,
                    tc.tile_pool(name="bpsum", bufs=2, space="PSUM") as bpsum,
                    tc.tile_pool(name="bmisc", bufs=1, space="PSUM") as bmisc,
                ):
                    qdn_all = bqp.tile([128, NLT, N_CORES, SHW], bf16)
                    qdn_src = qdn_full.rearrange(
                        "(r li p) s -> p li r s", p=128, li=NLT
                    )
                    for li in range(NLT):
                        nc.gpsimd.dma_start(
                            qdn_all[:, li, :, :], qdn_src[:, li, :, :]
                        )
                    for c in range(NC_):
                        cs = slice(CW * c, CW * (c + 1))
                        for dt in range(3):
                            ps_qt = bpsum.tile([128, CW], f32, tag="qt")
                            for li in range(NLT):
                                nc.tensor.matmul(
                                    ps_qt,
                                    wqb_sb[:, li, 128 * dt : 128 * (dt + 1)],
                                    qdn_all[:, li, 2 * c : 2 * c + 2, :],
                                    start=(li == 0),
                                    stop=(li == NLT - 1),
                                )
                            nc.vector.tensor_copy(qtr[:, dt, cs], ps_qt)
                        # rope q_pe (both heads stacked)
                        ps_sw = bmisc.tile([128, CW], f32, tag="misc",
                                           name="ps_swq")
                        nc.tensor.matmul(
                            ps_sw, swapp_sb, qtr[:, 2, cs], start=True, stop=True
                        )
                        t1 = bwork.tile([128, CW], f32, tag="t1")
                        nc.vector.tensor_tensor(
                            t1, qtr[:, 2, cs].bitcast(f32), cos2_sb[:, cs], OP.mult
                        )
                        t2 = bwork.tile([128, CW], f32, tag="t2")
                        nc.vector.tensor_tensor(t2, ps_sw, sin2n_sb[:, cs], OP.mult)
                        nc.vector.tensor_tensor(qspe[:, cs], t1, t2, OP.add)

                # ================= Phase A: attention =================
                with (
                    tc.tile_pool(name="asm", bufs=2) as asm,
                    tc.tile_pool(name="attp", bufs=3) as attp,
                    tc.tile_pool(name="aacc", bufs=1, space="PSUM") as aacc,
                    tc.tile_pool(name="ascore", bufs=2, space="PSUM") as ascore,
                    tc.tile_pool(name="ascr1", bufs=1, space="PSUM") as ascr1,
                    tc.tile_pool(name="amisc", bufs=1, space="PSUM") as amisc,
                ):
                    pending_epilogue = [None]

                    for c in range(NC_):
                        cs = slice(CW * c, CW * (c + 1))
                        nj = 4 * c + 4
                        ps_ctx = [
                            aacc.tile([128, CW], f32, tag=f"ctx{h}",
                                      name=f"ps_ctx{h}")
                            for h in range(H_PER_CORE)
                        ]
                        ps_sum = [
                            aacc.tile([1, CW], f32, tag=f"sum{h}",
                                      name=f"ps_sum{h}")
                            for h in range(H_PER_CORE)
                        ]
                        # software pipeline: scores(j) on PE, then ctx(j-1);
                        # exp(j) on ACT overlaps ctx(j-1)+scores(j+1) on PE.
                        atts = [[None, None] for _ in range(nj)]
                        offs = [0 if j < 4 * c else 128 * (j - 4 * c)
                                for j in range(nj)]

                        def emit_scores(j, c=c, nj=nj, offs=offs, atts=atts):
                            off = offs[j]
                            q0 = CW * c + off
                            qs = slice(q0, CW * (c + 1))
                            for h in range(H_PER_CORE):
                                pool = ascore if h == 0 else ascr1
                                ps_s = pool.tile([128, CW], f32, tag=f"sc{h}",
                                                 name=f"ps_s{h}")
                                nc.tensor.matmul(
                                    ps_s[:, off:],
                                    kn_sb[:, h, 128 * j : 128 * (j + 1)],
                                    qtr[:, h, qs],
                                    start=True,
                                    stop=False,
                                )
                                nc.tensor.matmul(
                                    ps_s[:, off:],
                                    kpe[64 * h : 64 * (h + 1),
                                        128 * j : 128 * (j + 1)],
                                    qspe[64 * h : 64 * (h + 1), qs],
                                    start=False,
                                    stop=True,
                                )
                                att = attp.tile([128, CW], f32r, tag=f"att{h}",
                                                name=f"att{h}")
                                nc.scalar.activation(
                                    att[:, off:], ps_s[:, off:], AF.Exp, scale=SCALE
                                )
                                if j >= 4 * c:
                                    nc.vector.tensor_tensor(
                                        att[:, off : off + 128],
                                        att[:, off : off + 128].bitcast(f32),
                                        maskt_sb,
                                        OP.mult,
                                    )
                                atts[j][h] = att

                        def emit_ctx(j, c=c, nj=nj, offs=offs, atts=atts,
                                     ps_ctx=ps_ctx, ps_sum=ps_sum):
                            off = offs[j]
                            for h in range(H_PER_CORE):
                                nc.tensor.matmul(
                                    ps_ctx[h][:, off:],
                                    v_sb[:, j, VD * h : VD * (h + 1)],
                                    atts[j][h][:, off:],
                                    start=(j == 0),
                                    stop=(j == nj - 1),
                                )
                                nc.tensor.matmul(
                                    ps_sum[h][:, off:],
                                    ones_p,
                                    atts[j][h][:, off:],
                                    start=(j == 0),
                                    stop=(j == nj - 1),
                                )

                        emit_scores(0)
                        emit_scores(1)
                        if pending_epilogue[0] is not None:
                            pending_epilogue[0]()
                        emit_ctx(0)
                        for j in range(2, nj):
                            emit_scores(j)
                            emit_ctx(j - 1)
                        emit_ctx(nj - 1)

                        def epilogue(c=c, cs=cs, ps_ctx=ps_ctx, ps_sum=ps_sum):
                            for h in range(H_PER_CORE):
                                s_sb = asm.tile([1, CW], f32, tag="ssb")
                                nc.vector.tensor_copy(s_sb, ps_sum[h])
                                nc.vector.reciprocal_approx_fast(
                                    out=s_sb, in_=s_sb
                                )
                                rs_r = asm.tile([1, CW], f32r, tag="rsr")
                                nc.vector.tensor_copy(rs_r, s_sb)
                                ps_rb2 = amisc.tile([128, CW], f32, tag="misc",
                                                    name="ps_rb2")
                                nc.tensor.matmul(
                                    ps_rb2, ones_row, rs_r, start=True, stop=True
                                )
                                rsb = asm.tile([128, CW], f32, tag="rsb")
                                nc.vector.tensor_copy(rsb, ps_rb2)
                                nc.vector.tensor_tensor(
                                    ctxa[:, h, cs], ps_ctx[h], rsb, OP.mult
                                )

                        pending_epilogue[0] = epilogue
                    pending_epilogue[0]()

                # ================= Phase W: output projection =================
                with (
                    tc.tile_pool(name="obp", bufs=3) as obp,
                    tc.tile_pool(name="wpsum", bufs=2, space="PSUM") as wpsum,
                ):
                    for c in range(NC_):
                        cs = slice(CW * c, CW * (c + 1))
                        for ht in range(KB):
                            ps_o = wpsum.tile([128, CW], f32, tag="o")
                            for h in range(H_PER_CORE):
                                nc.tensor.matmul(
                                    ps_o,
                                    wo_sb[:, h, 128 * ht : 128 * (ht + 1)],
                                    ctxa[:, h, cs],
                                    start=(h == 0),
                                    stop=(h == H_PER_CORE - 1),
                                )
                            ob = obp.tile([128, CW], bf16, tag="ob")
                            nc.vector.tensor_copy(ob, ps_o)
                            nc.sync.dma_start(
                                out_t.ap()[128 * ht : 128 * (ht + 1), cs], ob
                            )

    nc.finalize()
    return nc


_PROGRAM = None


def _get_program():
    global _PROGRAM
    if _PROGRAM is None:
        _PROGRAM = _build_program()
    return _PROGRAM


def _host_inputs(hidden_states, position_ids, wq_a, q_a_ln_w, wq_b, wkv_a,
                 kv_a_ln_w, wkv_b, wo):
    """Build the 8 per-core input maps."""
    hs = np.asarray(hidden_states, np.float32)[0]          # [S, HID]
    pos = np.asarray(position_ids)[0].astype(np.int64)     # [S]

    # rope tables (fp32, matching the reference)
    inv_freq = (1.0 / (THETA ** (np.arange(0, ROPE, 2, dtype=np.float32) / ROPE))).astype(np.float32)
    t = pos.astype(np.float32)
    freqs = np.outer(t, inv_freq).astype(np.float32)       # [S, 32]
    emb = np.concatenate([freqs, freqs], -1)               # [S, 64]
    cos = np.cos(emb).astype(np.float32)
    sin = np.sin(emb).astype(np.float32)
    cosT = np.ascontiguousarray(cos.T)                     # [64, S]
    sinT = np.ascontiguousarray(sin.T)
    sinTn = sinT.copy()
    sinTn[:32] = -sinTn[:32]                               # fold rotate_half sign
    cos2 = np.concatenate([cosT, cosT], 0)                 # [128, S]
    sin2n = np.concatenate([sinTn, sinTn], 0)

    perm = np.concatenate([np.arange(0, ROPE, 2), np.arange(1, ROPE, 2)])  # interleave

    # swap-halves permutation matrix (two independent 64 blocks)
    swapp = np.zeros((128, 128), np.float32)
    for m in range(128):
        base = (m // 64) * 64
        i = m % 64
        swapp[base + (i + 32) % 64, m] = 1.0

    maskt = np.triu(np.ones((128, 128), np.float32))

    wq_b = np.asarray(wq_b, np.float32) * np.asarray(q_a_ln_w, np.float32)[None, :]
    kvb = np.asarray(wkv_b, np.float32).reshape(16, NOPE + VD, KVL)
    wkv_a = np.asarray(wkv_a, np.float32)
    wkv_rows = np.concatenate(
        [wkv_a[:KVL], wkv_a[KVL:][perm], wkv_a[KVL:][perm]], 0
    )                                                      # [640, HID]

    hid_T = np.ascontiguousarray(hs.T)                     # [HID, S]
    shared = {
        "hid_t": _bf16(hid_T),
        "wqa_t": _bf16(np.asarray(wq_a, np.float32).T),
        "wkv_t": _bf16(wkv_rows.T),
        "kvln": _tf32_rne(np.asarray(kv_a_ln_w, np.float32)[None, :]),
        "cos2": cos2, "sin2n": sin2n,
        "swapp": _tf32_rne(swapp), "maskt": maskt,
    }

    wo = np.asarray(wo, np.float32)
    in_maps = []
    for core in range(N_CORES):
        h0 = H_PER_CORE * core
        blocks = []
        pe_rows = []
        for h in (h0, h0 + 1):
            blk = wq_b[192 * h : 192 * (h + 1)]
            blocks.append(blk[:NOPE])
            pe_rows.append(blk[NOPE:][perm])
        wqb_re = np.concatenate(blocks + pe_rows, 0)       # [384, QLR]
        wukt = np.stack(
            [np.ascontiguousarray(kvb[h, :NOPE, :].T) for h in (h0, h0 + 1)]
        )                                                  # [2, 512, 128]
        wuv2 = np.concatenate(
            [kvb[h, NOPE:, :].T for h in (h0, h0 + 1)], axis=1
        )                                                  # [512, 256]
        wo_c = np.ascontiguousarray(wo[:, VD * h0 : VD * (h0 + 2)].T)   # [256, HID]
        in_maps.append({
            **shared,
            "hid_own": _bf16(hid_T[:, SHW * core : SHW * (core + 1)]),
            "wqb_t": _bf16(wqb_re.T),
            "wukt": _tf32_rne(wukt),
            "wuv2": _tf32_rne(np.ascontiguousarray(wuv2)),
            "wo_t": _tf32_rne(wo_c),
        })
    return in_maps


def kernel(**inputs):
    from concourse.bass_utils import run_bass_kernel_spmd

    nc = _get_program()
    in_maps = _host_inputs(**inputs)
    res = run_bass_kernel_spmd(nc, in_maps, core_ids=list(range(N_CORES)))
    acc = None
    for r in res.results:
        o = np.asarray(r["out_t"], dtype=np.float32)
        acc = o if acc is None else acc + o
    out = np.ascontiguousarray(acc.T)[None]                # [1, S, HID]
    return out.astype(np.float32)


# revision 32
# speedup vs baseline: 1.1392x; 1.0211x over previous
"""MLA (DeepSeek-style multi-head latent attention) forward on 8 TRN2 NeuronCores.

Sharding: the q_down projection (the largest replicated GEMM) is sharded over
sequence (each core computes its own 256 rows, normalized, bf16) and
AllGathered on-device while the still-replicated ckv projection and the K/V
materialization run on the tensor engine — the collective hides behind ~93us
of PE work. Attention and the output projection are tensor-parallel over
heads (16 heads -> 2 per core); partial wo outputs are summed on host.

Device layout is "feature-major" (features on SBUF partitions, sequence on the
free dim) throughout. Attention uses the prefill-optimal NON-absorbed form:
per-head K (128-dim nope) and V (128-dim) are materialized from the shared
latent once, so scores contract over 192 dims (not 576) and ctx over 128
(not 512). Scores come out k-major ([k, q]); softmax normalization over k is
done with ones-matmuls on the tensor engine.

The projections run with bf16 inputs and weights (half the DMA, same PE rate
as fp32r at free-dim >= 256), accumulating in fp32 PSUM. Attention operands
stay float32r (TF32). wo partials return in bf16 and are summed in fp32 on
host.

Pipeline per core (S=2048; 4 seq-chunks of 512 for full-S phases):
  L:  local q_down shard: q_downT[:, own 256 cols] = wq_a.T^T @ hid_ownT,
      software-pipelined li-groups (sum-of-squares via ACT Square straight
      from PSUM + ones-matmul trail one group behind); rms fold: shard is
      normalized (r_q broadcast via gpsimd) and rounded to bf16; DMA to DRAM
      -> AllGather across the 8 cores.
  C:  ckvT = wkv_a'.T^T @ hidT replicated (RoPE interleave baked into the pe
      rows of wkv_a) -> DRAM spill. Runs while the AllGather flies.
  K:  two-stage pipeline per chunk: latent rms (kv ln folded into the
      broadcast matmul), then per-head k_nopeT (feature-major) and V
      (seq-major, both heads side by side) for the previous chunk; RoPE k_pe.
  B:  post-gather wq_b: qT (all heads' rows for this core) from the gathered
      normalized q_down, PSUM-accumulated over the 12 l-tiles; RoPE q_pe.
  A:  per k-block, software-pipelined over both heads: scoresT -> exp (no
      max subtraction needed: |score*scale| <= ~4) -> causal mask on the
      diagonal block (suffix-sliced matmuls skip fully-masked columns) ->
      ctxT + softmax denominator accumulation in PSUM; per-head epilogues
      deferred into the next chunk's score stream.
  W:  wo partial matmul -> bf16 DRAM outT.
Host: sum the 8 partial outT in fp32, transpose -> [1, S, HID].
"""

import numpy as np

S = 2048
HID = 2048
QLR = 1536
H_PER_CORE = 2
N_CORES = 8
NOPE = 128
ROPE = 64
VD = 128
KVL = 512
EPS = 1e-6
THETA = 10000.0
SCALE = float((NOPE + ROPE) ** -0.5)
NC_ = 4            # seq chunks
CW = 512           # chunk width
SHW = S // N_CORES  # 256-wide local shard
KB = S // 128      # 16 k-blocks
NLT = QLR // 128   # 12 l-tiles


def _tf32_rne(a):
    a = np.ascontiguousarray(a, dtype=np.float32)
    u = a.view(np.uint32).astype(np.uint64)
    u = (u + 0xFFF + ((u >> 13) & 1)) & 0xFFFFE000
    return u.astype(np.uint32).view(np.float32)


def _bf16(a):
    import ml_dtypes
    return np.ascontiguousarray(np.asarray(a, np.float32)).astype(ml_dtypes.bfloat16)


def _build_program():
    import concourse.mybir as mybir
    import concourse.tile as tile
    import concourse.bass_isa as bass_isa
    from concourse import bacc

    f32 = mybir.dt.float32
    f32r = mybir.dt.float32r
    bf16 = mybir.dt.bfloat16
    AF = mybir.ActivationFunctionType
    OP = mybir.AluOpType

    nc = bacc.Bacc("TRN2", target_bir_lowering=False, num_devices=N_CORES)

    hid_own = nc.dram_tensor("hid_own", [HID, SHW], bf16, kind="ExternalInput")
    hid_t = nc.dram_tensor("hid_t", [HID, S], bf16, kind="ExternalInput")
    wqa_t = nc.dram_tensor("wqa_t", [HID, QLR], bf16, kind="ExternalInput")
    wqb_t = nc.dram_tensor("wqb_t", [QLR, 384], bf16, kind="ExternalInput")
    wkv_t = nc.dram_tensor("wkv_t", [HID, 640], bf16, kind="ExternalInput")
    kvln_d = nc.dram_tensor("kvln", [1, KVL], f32r, kind="ExternalInput")
    wukt_d = nc.dram_tensor("wukt", [H_PER_CORE, KVL, NOPE], f32r,
                            kind="ExternalInput")
    wuv2_d = nc.dram_tensor("wuv2", [KVL, H_PER_CORE * VD], f32r,
                            kind="ExternalInput")
    wo_t = nc.dram_tensor("wo_t", [H_PER_CORE * VD, HID], f32r, kind="ExternalInput")
    cos2_d = nc.dram_tensor("cos2", [128, S], f32, kind="ExternalInput")
    sin2n_d = nc.dram_tensor("sin2n", [128, S], f32, kind="ExternalInput")
    swapp_d = nc.dram_tensor("swapp", [128, 128], f32r, kind="ExternalInput")
    maskt_d = nc.dram_tensor("maskt", [128, 128], bf16, kind="ExternalInput")
    out_t = nc.dram_tensor("out_t", [HID, S], bf16, kind="ExternalOutput")

    with tile.TileContext(nc) as tc:
        with (
            tc.tile_pool(name="stats", bufs=1) as stats,
            tc.tile_pool(name="dram", bufs=1, space="DRAM") as dram,
        ):
            ones_p = stats.tile([128, 1], f32r)
            nc.vector.memset(ones_p.bitcast(f32), 1.0)
            ones_row = stats.tile([1, 128], f32r)
            nc.vector.memset(ones_row.bitcast(f32), 1.0)
            eps_sb = stats.tile([1, 1], f32)
            nc.vector.memset(eps_sb, EPS)

            ckv_spill = dram.tile([128, 5, S], f32r)
            qdn_shard = dram.tile([QLR, SHW], bf16)
            qdn_full = dram.tile([N_CORES * QLR, SHW], bf16,
                                 addr_space="Shared")

            # ========== Phase L: local q_down shard -> AllGather ==========
            # hid / wkv for phase C are DMA'd up front: once the AllGather
            # launches it monopolizes the DMA engines, so everything phase C
            # needs must already be on-chip.
            hidp_ctx = tc.tile_pool(name="hidp", bufs=1)
            hidp = hidp_ctx.__enter__()
            wkvp_ctx = tc.tile_pool(name="wkvp", bufs=1)
            wkvp = wkvp_ctx.__enter__()
            with (
                tc.tile_pool(name="hidop", bufs=1) as hidop,
                tc.tile_pool(name="wqap", bufs=1) as wqap,
                tc.tile_pool(name="lwork", bufs=2) as lwork,
                tc.tile_pool(name="lbig", bufs=1) as lbig,
                tc.tile_pool(name="lpsum", bufs=2, space="PSUM") as lpsum,
                tc.tile_pool(name="lpsum1", bufs=1, space="PSUM") as lpsum1,
            ):
                hid_own_sb = hidop.tile([128, KB, SHW], bf16)
                for kt in range(KB):
                    nc.sync.dma_start(
                        hid_own_sb[:, kt, :],
                        hid_own.ap()[128 * kt : 128 * (kt + 1), :],
                    )
                wqa_sb = wqap.tile([128, KB, QLR], bf16)
                for lg in range(3):
                    nc.sync.dma_start(
                        wqa_sb[:, :, 512 * lg : 512 * (lg + 1)],
                        wqa_t.ap()[:, 512 * lg : 512 * (lg + 1)].rearrange(
                            "(kt p) m -> p kt m", p=128
                        ),
                    )
                hid_sb = hidp.tile([128, KB, S], bf16)
                for kt in range(KB):
                    nc.sync.dma_start(
                        hid_sb[:, kt, :], hid_t.ap()[128 * kt : 128 * (kt + 1), :]
                    )
                wkv_sb = wkvp.tile([128, KB, 640], bf16, tag="wkv")
                for dt in range(5):
                    nc.sync.dma_start(
                        wkv_sb[:, :, 128 * dt : 128 * (dt + 1)],
                        wkv_t.ap()[:, 128 * dt : 128 * (dt + 1)].rearrange(
                            "(kt p) m -> p kt m", p=128
                        ),
                    )
                qdraw = lbig.tile([128, NLT, SHW], f32)
                qdn_sb = lbig.tile([128, NLT, SHW], bf16)
                ssql = stats.tile([1, SHW], f32)
                rqbl = stats.tile([128, SHW], f32)
                ps_ssq = lpsum1.tile([1, SHW], f32, tag="ssq")

                prev_qd = [None]

                def emit_ssq(li):
                    sq = lwork.tile([128, SHW], f32r, tag="sq")
                    nc.scalar.activation(sq, prev_qd[0], AF.Square)
                    nc.tensor.matmul(
                        ps_ssq, ones_p, sq, start=(li == 0), stop=(li == NLT - 1)
                    )

                for li in range(NLT):
                    ps_qd = lpsum.tile([128, SHW], f32, tag="qd",
                                       name=f"ps_qd{li % 2}")
                    for kt in range(KB):
                        nc.tensor.matmul(
                            ps_qd,
                            wqa_sb[:, kt, 128 * li : 128 * (li + 1)],
                            hid_own_sb[:, kt, :],
                            start=(kt == 0),
                            stop=(kt == KB - 1),
                        )
                    if li > 0:
                        emit_ssq(li - 1)
                    nc.vector.tensor_copy(qdraw[:, li, :], ps_qd)
                    prev_qd[0] = ps_qd
                emit_ssq(NLT - 1)
                nc.scalar.activation(
                    ssql, ps_ssq, AF.Sqrt, scale=1.0 / QLR, bias=eps_sb
                )
                nc.vector.reciprocal_approx_fast(out=ssql, in_=ssql)
                nc.gpsimd.partition_broadcast(rqbl, ssql, channels=128)
                for li in range(NLT):
                    nc.vector.tensor_tensor(
                        qdn_sb[:, li, :], qdraw[:, li, :], rqbl, OP.mult
                    )
                nc.gpsimd.dma_start(
                    qdn_shard.rearrange("(li p) s -> p li s", p=128),
                    qdn_sb,
                )
                nc.gpsimd.collective_compute(
                    "AllGather",
                    mybir.AluOpType.bypass,
                    replica_groups=[list(range(N_CORES))],
                    ins=[qdn_shard.opt()],
                    outs=[qdn_full.opt()],
                )

            # ========== Phase C: replicated ckvT (overlaps the AllGather) =====
            with (
                tc.tile_pool(name="cwork", bufs=2) as cwork,
                tc.tile_pool(name="cpsum", bufs=2, space="PSUM") as cpsum,
            ):
                for c in range(NC_):
                    cs = slice(CW * c, CW * (c + 1))
                    for dt in range(5):
                        ps_ck = cpsum.tile([128, CW], f32, tag="ck")
                        for kt in range(KB):
                            nc.tensor.matmul(
                                ps_ck,
                                wkv_sb[:, kt, 128 * dt : 128 * (dt + 1)],
                                hid_sb[:, kt, cs],
                                start=(kt == 0),
                                stop=(kt == KB - 1),
                            )
                        ckb = cwork.tile([128, CW], f32r, tag="ckb")
                        nc.vector.tensor_copy(ckb, ps_ck)
                        nc.sync.dma_start(ckv_spill[:, dt, cs], ckb)
            wkvp_ctx.__exit__(None, None, None)
            hidp_ctx.__exit__(None, None, None)

            # ============ late constants + persistent attention tensors ============
            with (
                tc.tile_pool(name="consts", bufs=1) as consts,
                tc.tile_pool(name="resid", bufs=1) as resid,
            ):
                kvln_sb = consts.tile([1, KVL], f32r)
                nc.sync.dma_start(kvln_sb, kvln_d.ap())
                wukt_sb = consts.tile([128, H_PER_CORE, 4, NOPE], f32r)
                nc.sync.dma_start(
                    wukt_sb, wukt_d.ap().rearrange("h (lt p) n -> p h lt n", p=128)
                )
                wuv2_sb = consts.tile([128, 4, H_PER_CORE * VD], f32r)
                nc.sync.dma_start(
                    wuv2_sb, wuv2_d.ap().rearrange("(lt p) v -> p lt v", p=128)
                )
                cos2_sb = consts.tile([128, S], f32)
                nc.sync.dma_start(cos2_sb, cos2_d.ap())
                sin2n_sb = consts.tile([128, S], f32)
                nc.sync.dma_start(sin2n_sb, sin2n_d.ap())
                swapp_sb = consts.tile([128, 128], f32r)
                nc.sync.dma_start(swapp_sb, swapp_d.ap())
                maskt_sb = consts.tile([128, 128], bf16)
                nc.sync.dma_start(maskt_sb, maskt_d.ap())

                kpe = resid.tile([128, S], f32r)          # roped k_peT (2 head copies)
                kn_sb = resid.tile([128, H_PER_CORE, S], f32r)  # per-head k_nopeT
                v_sb = resid.tile([128, KB, H_PER_CORE * VD], bf16)  # V seq-major
                ctxa = resid.tile([128, H_PER_CORE, S], f32r)
                wo_sb = resid.tile([128, H_PER_CORE, HID], f32r)
                nc.sync.dma_start(
                    wo_sb, wo_t.ap().rearrange("(h p) m -> p h m", p=128)
                )
                qtr = resid.tile([128, 3, S], f32r)       # post-gather q (r_q folded)
                qspe = resid.tile([128, S], f32r)         # roped q_peT
                wqb_sb = resid.tile([128, NLT, 384], bf16)
                nc.sync.dma_start(
                    wqb_sb,
                    wqb_t.ap().rearrange("(li p) m -> p li m", p=128),
                )

                # ===== Phase K: latent rms + per-head K/V materialization =====
                # Two-stage pipeline: rms chain of chunk c overlaps the
                # materialization matmuls of chunk c-1 on the PE.
                with (
                    tc.tile_pool(name="kwork", bufs=2) as kwork,
                    tc.tile_pool(name="kpsum", bufs=2, space="PSUM") as kpsum,
                    tc.tile_pool(name="kpsum1", bufs=1, space="PSUM") as kpsum1,
                ):
                    ksn_tiles = [None] * NC_
                    ck_tiles = [None] * NC_
                    rk_tiles = [None] * NC_

                    def emit_rms_a(c):
                        cs = slice(CW * c, CW * (c + 1))
                        ck = kwork.tile([128, 5, CW], f32r, tag="ck",
                                        name=f"ck{c}")
                        nc.sync.dma_start(ck, ckv_spill[:, :, cs])
                        ps_ssqk = kpsum1.tile([1, CW], f32, tag="ssqk")
                        for j in range(4):
                            sqk = kwork.tile([128, CW], f32r, tag="sqk")
                            nc.scalar.activation(
                                sqk, ck[:, j, :].bitcast(f32), AF.Square
                            )
                            nc.tensor.matmul(
                                ps_ssqk, ones_p, sqk, start=(j == 0), stop=(j == 3)
                            )
                        rk = kwork.tile([1, CW], f32, tag="rk")
                        nc.scalar.activation(
                            rk, ps_ssqk, AF.Sqrt, scale=1.0 / KVL, bias=eps_sb
                        )
                        nc.vector.reciprocal_approx_fast(out=rk, in_=rk)
                        rk_r = kwork.tile([1, CW], f32r, tag="rkr")
                        nc.vector.tensor_copy(rk_r, rk)
                        ck_tiles[c] = ck
                        rk_tiles[c] = rk_r

                    def emit_rms_b(c):
                        ck = ck_tiles[c]
                        rk_r = rk_tiles[c]
                        ksn_c = kwork.tile([128, 4, CW], f32r, tag="ksn",
                                           name=f"ksn{c}")
                        for j in range(4):
                            ps_b = kpsum1.tile([128, CW], f32, tag="bc")
                            nc.tensor.matmul(
                                ps_b,
                                kvln_sb[0:1, 128 * j : 128 * (j + 1)],
                                rk_r,
                                start=True,
                                stop=True,
                            )
                            nc.vector.tensor_tensor(
                                ksn_c[:, j, :], ck[:, j, :].bitcast(f32), ps_b,
                                OP.mult
                            )
                        ksn_tiles[c] = ksn_c

                    def emit_mat(c):
                        cs = slice(CW * c, CW * (c + 1))
                        ksn_c = ksn_tiles[c]
                        ck = ck_tiles[c]
                        for h in range(H_PER_CORE):
                            ps_k = kpsum.tile([128, CW], f32, tag="kn")
                            for lt in range(4):
                                nc.tensor.matmul(
                                    ps_k,
                                    wukt_sb[:, h, lt, :],
                                    ksn_c[:, lt, :],
                                    start=(lt == 0),
                                    stop=(lt == 3),
                                )
                            nc.vector.tensor_copy(kn_sb[:, h, cs], ps_k)
                        for b in range(4):
                            ps_v = kpsum.tile([128, H_PER_CORE * VD], f32, tag="v")
                            for lt in range(4):
                                nc.tensor.matmul(
                                    ps_v,
                                    ksn_c[:, lt, 128 * b : 128 * (b + 1)],
                                    wuv2_sb[:, lt, :],
                                    start=(lt == 0),
                                    stop=(lt == 3),
                                )
                            nc.vector.tensor_copy(v_sb[:, 4 * c + b, :], ps_v)
                        # k_pe rope (both 64-row copies at once)
                        ps_sw = kpsum1.tile([128, CW], f32, tag="sw")
                        nc.tensor.matmul(
                            ps_sw, swapp_sb, ck[:, 4, :], start=True, stop=True
                        )
                        t1 = kwork.tile([128, CW], f32, tag="t1")
                        nc.vector.tensor_tensor(
                            t1, ck[:, 4, :].bitcast(f32), cos2_sb[:, cs], OP.mult
                        )
                        t2 = kwork.tile([128, CW], f32, tag="t2")
                        nc.vector.tensor_tensor(t2, ps_sw, sin2n_sb[:, cs], OP.mult)
                        nc.vector.tensor_tensor(kpe[:, cs], t1, t2, OP.add)

                    emit_rms_a(0)
                    emit_rms_b(0)
                    for c in range(1, NC_):
                        emit_rms_a(c)
                        emit_mat(c - 1)
                        emit_rms_b(c)
                    emit_mat(NC_ - 1)

                # ===== Phase B: post-gather wq_b + q rope =====
                with (
                    tc.tile_pool(name="bqp", bufs=1) as bqp,
                    tc.tile_pool(name="bwork", bufs=2) as bwork,
                    tc.tile_pool(name="bpsum", bufs=2, space="PSUM") as bpsum,
                    tc.tile_pool(name="bmisc", bufs=1, space="PSUM") as bmisc,
                ):
                    qdn_all = bqp.tile([128, NLT, N_CORES, SHW], bf16)
                    qdn_src = qdn_full.rearrange(
                        "(r li p) s -> p li r s", p=128, li=NLT
                    )
                    for li in range(NLT):
                        nc.gpsimd.dma_start(
                            qdn_all[:, li, :, :], qdn_src[:, li, :, :]
                        )
                    for c in range(NC_):
                        cs = slice(CW * c, CW * (c + 1))
                        for dt in range(3):
                            ps_qt = bpsum.tile([128, CW], f32, tag="qt")
                            for li in range(NLT):
                                nc.tensor.matmul(
                                    ps_qt,
                                    wqb_sb[:, li, 128 * dt : 128 * (dt + 1)],
                                    qdn_all[:, li, 2 * c : 2 * c + 2, :],
                                    start=(li == 0),
                                    stop=(li == NLT - 1),
                                )
                            nc.vector.tensor_copy(qtr[:, dt, cs], ps_qt)
                        # rope q_pe (both heads stacked)
                        ps_sw = bmisc.tile([128, CW], f32, tag="misc",
                                           name="ps_swq")
                        nc.tensor.matmul(
                            ps_sw, swapp_sb, qtr[:, 2, cs], start=True, stop=True
                        )
                        t1 = bwork.tile([128, CW], f32, tag="t1")
                        nc.vector.tensor_tensor(
                            t1, qtr[:, 2, cs].bitcast(f32), cos2_sb[:, cs], OP.mult
                        )
                        t2 = bwork.tile([128, CW], f32, tag="t2")
                        nc.vector.tensor_tensor(t2, ps_sw, sin2n_sb[:, cs], OP.mult)
                        nc.vector.tensor_tensor(qspe[:, cs], t1, t2, OP.add)

                # ================= Phase A: attention =================
                with (
                    tc.tile_pool(name="accp", bufs=2) as accp,
                    tc.tile_pool(name="attp", bufs=3) as attp,
                    tc.tile_pool(name="aacc", bufs=1, space="PSUM") as aacc,
                    tc.tile_pool(name="ascore", bufs=2, space="PSUM") as ascore,
                    tc.tile_pool(name="ascr1", bufs=2, space="PSUM") as ascr1,
                ):
                    pending_epilogue = [None]

                    for c in range(NC_):
                        cs = slice(CW * c, CW * (c + 1))
                        nj = 4 * c + 4
                        ps_ctx = [
                            aacc.tile([128, CW], f32, tag=f"ctx{h}",
                                      name=f"ps_ctx{h}")
                            for h in range(H_PER_CORE)
                        ]
                        dacc = [
                            accp.tile([128, CW], f32, tag=f"dacc{h}",
                                      name=f"dacc{h}")
                            for h in range(H_PER_CORE)
                        ]
                        # software pipeline: scores(j) on PE, then ctx(j-1);
                        # exp(j) on ACT overlaps ctx(j-1)+scores(j+1) on PE.
                        # Softmax denominators accumulate on DVE into dacc and
                        # reduce across partitions on gpsimd -- no PE work.
                        atts = [[None, None] for _ in range(nj)]
                        offs = [0 if j < 4 * c else 128 * (j - 4 * c)
                                for j in range(nj)]

                        def emit_scores(j, c=c, nj=nj, offs=offs, atts=atts,
                                        dacc=dacc):
                            off = offs[j]
                            q0 = CW * c + off
                            qs = slice(q0, CW * (c + 1))
                            for h in range(H_PER_CORE):
                                pool = ascore if h == 0 else ascr1
                                ps_s = pool.tile([128, CW], f32, tag=f"sc{h}",
                                                 name=f"ps_s{h}")
                                nc.tensor.matmul(
                                    ps_s[:, off:],
                                    kn_sb[:, h, 128 * j : 128 * (j + 1)],
                                    qtr[:, h, qs],
                                    start=True,
                                    stop=False,
                                )
                                nc.tensor.matmul(
                                    ps_s[:, off:],
                                    kpe[64 * h : 64 * (h + 1),
                                        128 * j : 128 * (j + 1)],
                                    qspe[64 * h : 64 * (h + 1), qs],
                                    start=False,
                                    stop=True,
                                )
                                att = attp.tile([128, CW], bf16, tag=f"att{h}",
                                                name=f"att{h}")
                                nc.scalar.activation(
                                    att[:, off:], ps_s[:, off:], AF.Exp, scale=SCALE
                                )
                                if j >= 4 * c:
                                    nc.vector.tensor_tensor(
                                        att[:, off : off + 128],
                                        att[:, off : off + 128],
                                        maskt_sb,
                                        OP.mult,
                                    )
                                if j == 0:
                                    nc.vector.tensor_copy(dacc[h], att)
                                else:
                                    nc.vector.tensor_tensor(
                                        dacc[h][:, off:],
                                        dacc[h][:, off:],
                                        att[:, off:],
                                        OP.add,
                                    )
                                atts[j][h] = att

                        def emit_ctx(j, c=c, nj=nj, offs=offs, atts=atts,
                                     ps_ctx=ps_ctx):
                            off = offs[j]
                            for h in range(H_PER_CORE):
                                nc.tensor.matmul(
                                    ps_ctx[h][:, off:],
                                    v_sb[:, j, VD * h : VD * (h + 1)],
                                    atts[j][h][:, off:],
                                    start=(j == 0),
                                    stop=(j == nj - 1),
                                )

                        emit_scores(0)
                        emit_scores(1)
                        if pending_epilogue[0] is not None:
                            pending_epilogue[0]()
                        emit_ctx(0)
                        for j in range(2, nj):
                            emit_scores(j)
                            emit_ctx(j - 1)
                        emit_ctx(nj - 1)
                        for h in range(H_PER_CORE):
                            nc.gpsimd.partition_all_reduce(
                                dacc[h], dacc[h], channels=128,
                                reduce_op=bass_isa.ReduceOp.add,
                            )
                            nc.vector.reciprocal_approx_fast(
                                out=dacc[h], in_=dacc[h]
                            )

                        def epilogue(c=c, cs=cs, ps_ctx=ps_ctx, dacc=dacc):
                            for h in range(H_PER_CORE):
                                nc.vector.tensor_tensor(
                                    ctxa[:, h, cs], ps_ctx[h], dacc[h], OP.mult
                                )

                        pending_epilogue[0] = epilogue
                    pending_epilogue[0]()

                # ================= Phase W: output projection =================
                with (
                    tc.tile_pool(name="obp", bufs=3) as obp,
                    tc.tile_pool(name="wpsum", bufs=2, space="PSUM") as wpsum,
                ):
                    for c in range(NC_):
                        cs = slice(CW * c, CW * (c + 1))
                        for ht in range(KB):
                            ps_o = wpsum.tile([128, CW], f32, tag="o")
                            for h in range(H_PER_CORE):
                                nc.tensor.matmul(
                                    ps_o,
                                    wo_sb[:, h, 128 * ht : 128 * (ht + 1)],
                                    ctxa[:, h, cs],
                                    start=(h == 0),
                                    stop=(h == H_PER_CORE - 1),
                                )
                            ob = obp.tile([128, CW], bf16, tag="ob")
                            nc.vector.tensor_copy(ob, ps_o)
                            nc.sync.dma_start(
                                out_t.ap()[128 * ht : 128 * (ht + 1), cs], ob
                            )

    nc.finalize()
    return nc


_PROGRAM = None


def _get_program():
    global _PROGRAM
    if _PROGRAM is None:
        _PROGRAM = _build_program()
    return _PROGRAM


def _host_inputs(hidden_states, position_ids, wq_a, q_a_ln_w, wq_b, wkv_a,
                 kv_a_ln_w, wkv_b, wo):
    """Build the 8 per-core input maps."""
    hs = np.asarray(hidden_states, np.float32)[0]          # [S, HID]
    pos = np.asarray(position_ids)[0].astype(np.int64)     # [S]

    # rope tables (fp32, matching the reference)
    inv_freq = (1.0 / (THETA ** (np.arange(0, ROPE, 2, dtype=np.float32) / ROPE))).astype(np.float32)
    t = pos.astype(np.float32)
    freqs = np.outer(t, inv_freq).astype(np.float32)       # [S, 32]
    emb = np.concatenate([freqs, freqs], -1)               # [S, 64]
    cos = np.cos(emb).astype(np.float32)
    sin = np.sin(emb).astype(np.float32)
    cosT = np.ascontiguousarray(cos.T)                     # [64, S]
    sinT = np.ascontiguousarray(sin.T)
    sinTn = sinT.copy()
    sinTn[:32] = -sinTn[:32]                               # fold rotate_half sign
    cos2 = np.concatenate([cosT, cosT], 0)                 # [128, S]
    sin2n = np.concatenate([sinTn, sinTn], 0)

    perm = np.concatenate([np.arange(0, ROPE, 2), np.arange(1, ROPE, 2)])  # interleave

    # swap-halves permutation matrix (two independent 64 blocks)
    swapp = np.zeros((128, 128), np.float32)
    for m in range(128):
        base = (m // 64) * 64
        i = m % 64
        swapp[base + (i + 32) % 64, m] = 1.0

    maskt = np.triu(np.ones((128, 128), np.float32))

    wq_b = np.asarray(wq_b, np.float32) * np.asarray(q_a_ln_w, np.float32)[None, :]
    kvb = np.asarray(wkv_b, np.float32).reshape(16, NOPE + VD, KVL)
    wkv_a = np.asarray(wkv_a, np.float32)
    wkv_rows = np.concatenate(
        [wkv_a[:KVL], wkv_a[KVL:][perm], wkv_a[KVL:][perm]], 0
    )                                                      # [640, HID]

    hid_T = np.ascontiguousarray(hs.T)                     # [HID, S]
    shared = {
        "hid_t": _bf16(hid_T),
        "wqa_t": _bf16(np.asarray(wq_a, np.float32).T),
        "wkv_t": _bf16(wkv_rows.T),
        "kvln": _tf32_rne(np.asarray(kv_a_ln_w, np.float32)[None, :]),
        "cos2": cos2, "sin2n": sin2n,
        "swapp": _tf32_rne(swapp), "maskt": _bf16(maskt),
    }

    wo = np.asarray(wo, np.float32)
    in_maps = []
    for core in range(N_CORES):
        h0 = H_PER_CORE * core
        blocks = []
        pe_rows = []
        for h in (h0, h0 + 1):
            blk = wq_b[192 * h : 192 * (h + 1)]
            blocks.append(blk[:NOPE])
            pe_rows.append(blk[NOPE:][perm])
        wqb_re = np.concatenate(blocks + pe_rows, 0)       # [384, QLR]
        wukt = np.stack(
            [np.ascontiguousarray(kvb[h, :NOPE, :].T) for h in (h0, h0 + 1)]
        )                                                  # [2, 512, 128]
        wuv2 = np.concatenate(
            [kvb[h, NOPE:, :].T for h in (h0, h0 + 1)], axis=1
        )                                                  # [512, 256]
        wo_c = np.ascontiguousarray(wo[:, VD * h0 : VD * (h0 + 2)].T)   # [256, HID]
        in_maps.append({
            **shared,
            "hid_own": _bf16(hid_T[:, SHW * core : SHW * (core + 1)]),
            "wqb_t": _bf16(wqb_re.T),
            "wukt": _tf32_rne(wukt),
            "wuv2": _tf32_rne(np.ascontiguousarray(wuv2)),
            "wo_t": _tf32_rne(wo_c),
        })
    return in_maps


def kernel(**inputs):
    from concourse.bass_utils import run_bass_kernel_spmd

    nc = _get_program()
    in_maps = _host_inputs(**inputs)
    res = run_bass_kernel_spmd(nc, in_maps, core_ids=list(range(N_CORES)))
    acc = None
    for r in res.results:
        o = np.asarray(r["out_t"], dtype=np.float32)
        acc = o if acc is None else acc + o
    out = np.ascontiguousarray(acc.T)[None]                # [1, S, HID]
    return out.astype(np.float32)


# revision 48
# speedup vs baseline: 1.2088x; 1.0611x over previous
"""MLA (DeepSeek-style multi-head latent attention) forward on 8 TRN2 NeuronCores.

Sharding: the q_down projection (the largest replicated GEMM) is sharded over
sequence (each core computes its own 256 rows, normalized, bf16) and
AllGathered on-device while the still-replicated ckv projection and the K/V
materialization run on the tensor engine — the collective hides behind ~93us
of PE work. Attention and the output projection are tensor-parallel over
heads (16 heads -> 2 per core); partial wo outputs are summed on host.

Device layout is "feature-major" (features on SBUF partitions, sequence on the
free dim) throughout. Attention uses the prefill-optimal NON-absorbed form:
per-head K (128-dim nope) and V (128-dim) are materialized from the shared
latent once, so scores contract over 192 dims (not 576) and ctx over 128
(not 512). Scores come out k-major ([k, q]); softmax normalization over k is
done with ones-matmuls on the tensor engine.

The projections run with bf16 inputs and weights (half the DMA, same PE rate
as fp32r at free-dim >= 256), accumulating in fp32 PSUM. Attention operands
stay float32r (TF32). wo partials return in bf16 and are summed in fp32 on
host.

Pipeline per core (S=2048; 4 seq-chunks of 512 for full-S phases):
  L:  local q_down shard: q_downT[:, own 256 cols] = wq_a.T^T @ hid_ownT,
      software-pipelined li-groups (sum-of-squares via ACT Square straight
      from PSUM + ones-matmul trail one group behind); rms fold: shard is
      normalized (r_q broadcast via gpsimd) and rounded to bf16; DMA to DRAM
      -> AllGather across the 8 cores.
  C:  ckvT = wkv_a'.T^T @ hidT replicated (RoPE interleave baked into the pe
      rows of wkv_a) -> DRAM spill. Runs while the AllGather flies.
  K:  two-stage pipeline per chunk: latent rms (kv ln folded into the
      broadcast matmul), then per-head k_nopeT (feature-major) and V
      (seq-major, both heads side by side) for the previous chunk; RoPE k_pe.
  B:  post-gather wq_b: qT (all heads' rows for this core) from the gathered
      normalized q_down, PSUM-accumulated over the 12 l-tiles; RoPE q_pe.
  A:  per k-block, software-pipelined over both heads: scoresT -> exp (no
      max subtraction needed: |score*scale| <= ~4) -> causal mask on the
      diagonal block (suffix-sliced matmuls skip fully-masked columns) ->
      ctxT + softmax denominator accumulation in PSUM; per-head epilogues
      deferred into the next chunk's score stream.
  W:  wo partial matmul -> bf16 DRAM outT.
Host: sum the 8 partial outT in fp32, transpose -> [1, S, HID].
"""

import numpy as np

S = 2048
HID = 2048
QLR = 1536
H_PER_CORE = 2
N_CORES = 8
NOPE = 128
ROPE = 64
VD = 128
KVL = 512
EPS = 1e-6
THETA = 10000.0
SCALE = float((NOPE + ROPE) ** -0.5)
NC_ = 4            # seq chunks
CW = 512           # chunk width
SHW = S // N_CORES  # 256-wide local shard
KB = S // 128      # 16 k-blocks
NLT = QLR // 128   # 12 l-tiles


def _tf32_rne(a):
    a = np.ascontiguousarray(a, dtype=np.float32)
    u = a.view(np.uint32).astype(np.uint64)
    u = (u + 0xFFF + ((u >> 13) & 1)) & 0xFFFFE000
    return u.astype(np.uint32).view(np.float32)


def _bf16(a):
    import ml_dtypes
    return np.ascontiguousarray(np.asarray(a, np.float32)).astype(ml_dtypes.bfloat16)


def _build_program():
    import concourse.mybir as mybir
    import concourse.tile as tile
    import concourse.bass_isa as bass_isa
    from concourse import bacc

    f32 = mybir.dt.float32
    f32r = mybir.dt.float32r
    bf16 = mybir.dt.bfloat16
    AF = mybir.ActivationFunctionType
    OP = mybir.AluOpType

    nc = bacc.Bacc("TRN2", target_bir_lowering=False, num_devices=N_CORES)

    hid_own = nc.dram_tensor("hid_own", [HID, SHW], bf16, kind="ExternalInput")
    hid_t = nc.dram_tensor("hid_t", [HID, S], bf16, kind="ExternalInput")
    wqa_t = nc.dram_tensor("wqa_t", [HID, QLR], bf16, kind="ExternalInput")
    wqb_t = nc.dram_tensor("wqb_t", [QLR, 384], bf16, kind="ExternalInput")
    wkv_t = nc.dram_tensor("wkv_t", [HID, 640], bf16, kind="ExternalInput")
    kvln_d = nc.dram_tensor("kvln", [1, KVL], f32r, kind="ExternalInput")
    wukt_d = nc.dram_tensor("wukt", [H_PER_CORE, KVL, NOPE], f32r,
                            kind="ExternalInput")
    wuv2_d = nc.dram_tensor("wuv2", [KVL, H_PER_CORE * VD], f32r,
                            kind="ExternalInput")
    wo_t = nc.dram_tensor("wo_t", [H_PER_CORE * VD, HID], f32r, kind="ExternalInput")
    cos2_d = nc.dram_tensor("cos2", [128, S], f32, kind="ExternalInput")
    sin2n_d = nc.dram_tensor("sin2n", [128, S], f32, kind="ExternalInput")
    swapp_d = nc.dram_tensor("swapp", [128, 128], f32r, kind="ExternalInput")
    swappb_d = nc.dram_tensor("swappb", [128, 128], bf16, kind="ExternalInput")
    maskt_d = nc.dram_tensor("maskt", [128, 128], bf16, kind="ExternalInput")
    out_t = nc.dram_tensor("out_t", [HID, S], bf16, kind="ExternalOutput")

    with tile.TileContext(nc) as tc:
        with (
            tc.tile_pool(name="stats", bufs=1) as stats,
            tc.tile_pool(name="dram", bufs=1, space="DRAM") as dram,
        ):
            ones_p = stats.tile([128, 1], f32r)
            nc.vector.memset(ones_p.bitcast(f32), 1.0)
            ones_row = stats.tile([1, 128], f32r)
            nc.vector.memset(ones_row.bitcast(f32), 1.0)
            eps_sb = stats.tile([1, 1], f32)
            nc.vector.memset(eps_sb, EPS)

            ckv_spill = dram.tile([128, 5, S], bf16)
            qdn_shard = dram.tile([QLR, SHW], bf16)
            qdn_full = dram.tile([N_CORES * QLR, SHW], bf16,
                                 addr_space="Shared")

            # ========== Phase L: local q_down shard -> AllGather ==========
            # hid / wkv for phase C are DMA'd up front: once the AllGather
            # launches it monopolizes the DMA engines, so everything phase C
            # needs must already be on-chip.
            hidp_ctx = tc.tile_pool(name="hidp", bufs=1)
            hidp = hidp_ctx.__enter__()
            wkvp_ctx = tc.tile_pool(name="wkvp", bufs=1)
            wkvp = wkvp_ctx.__enter__()
            with (
                tc.tile_pool(name="hidop", bufs=1) as hidop,
                tc.tile_pool(name="wqap", bufs=1) as wqap,
                tc.tile_pool(name="lwork", bufs=2) as lwork,
                tc.tile_pool(name="lbig", bufs=1) as lbig,
                tc.tile_pool(name="lpsum", bufs=2, space="PSUM") as lpsum,
                tc.tile_pool(name="lpsum1", bufs=1, space="PSUM") as lpsum1,
            ):
                hid_own_sb = hidop.tile([128, KB, SHW], bf16)
                for kt in range(KB):
                    nc.sync.dma_start(
                        hid_own_sb[:, kt, :],
                        hid_own.ap()[128 * kt : 128 * (kt + 1), :],
                    )
                wqa_sb = wqap.tile([128, KB, QLR], bf16)
                for lg in range(3):
                    nc.sync.dma_start(
                        wqa_sb[:, :, 512 * lg : 512 * (lg + 1)],
                        wqa_t.ap()[:, 512 * lg : 512 * (lg + 1)].rearrange(
                            "(kt p) m -> p kt m", p=128
                        ),
                    )
                wkv_sb = wkvp.tile([128, KB, 640], bf16, tag="wkv")
                for dt in range(5):
                    nc.sync.dma_start(
                        wkv_sb[:, :, 128 * dt : 128 * (dt + 1)],
                        wkv_t.ap()[:, 128 * dt : 128 * (dt + 1)].rearrange(
                            "(kt p) m -> p kt m", p=128
                        ),
                    )
                hid_sb = hidp.tile([128, KB, S], bf16)
                for kt in range(KB):
                    nc.sync.dma_start(
                        hid_sb[:, kt, :], hid_t.ap()[128 * kt : 128 * (kt + 1), :]
                    )
                qdraw = lbig.tile([128, NLT, SHW], f32)
                qdn_sb = lbig.tile([128, NLT, SHW], bf16)
                ssql = stats.tile([1, SHW], f32)
                rqbl = stats.tile([128, SHW], f32)
                ps_ssq = lpsum1.tile([1, SHW], f32, tag="ssq")

                prev_qd = [None]

                def emit_ssq(li):
                    sq = lwork.tile([128, SHW], f32r, tag="sq")
                    nc.scalar.activation(sq, prev_qd[0], AF.Square)
                    nc.tensor.matmul(
                        ps_ssq, ones_p, sq, start=(li == 0), stop=(li == NLT - 1)
                    )

                for li in range(NLT):
                    ps_qd = lpsum.tile([128, SHW], f32, tag="qd",
                                       name=f"ps_qd{li % 2}")
                    for kt in range(KB):
                        nc.tensor.matmul(
                            ps_qd,
                            wqa_sb[:, kt, 128 * li : 128 * (li + 1)],
                            hid_own_sb[:, kt, :],
                            start=(kt == 0),
                            stop=(kt == KB - 1),
                        )
                    if li > 0:
                        emit_ssq(li - 1)
                    nc.vector.tensor_copy(qdraw[:, li, :], ps_qd)
                    prev_qd[0] = ps_qd
                emit_ssq(NLT - 1)
                nc.scalar.activation(
                    ssql, ps_ssq, AF.Sqrt, scale=1.0 / QLR, bias=eps_sb
                )
                nc.vector.reciprocal_approx_fast(out=ssql, in_=ssql)
                nc.gpsimd.partition_broadcast(rqbl, ssql, channels=128)
                for li in range(NLT):
                    nc.vector.tensor_tensor(
                        qdn_sb[:, li, :], qdraw[:, li, :], rqbl, OP.mult
                    )
                nc.gpsimd.dma_start(
                    qdn_shard.rearrange("(li p) s -> p li s", p=128),
                    qdn_sb,
                )
                nc.gpsimd.collective_compute(
                    "AllGather",
                    mybir.AluOpType.bypass,
                    replica_groups=[list(range(N_CORES))],
                    ins=[qdn_shard.opt()],
                    outs=[qdn_full.opt()],
                )

            # ========== Phase C: replicated ckvT (overlaps the AllGather) =====
            # bf16 spill: the AllGather is saturating the DMA engines, so the
            # spill round-trip is kept as small as possible.
            with (
                tc.tile_pool(name="cwork", bufs=3) as cwork,
                tc.tile_pool(name="cpsum", bufs=2, space="PSUM") as cpsum,
            ):
                for c in range(NC_):
                    cs = slice(CW * c, CW * (c + 1))
                    for dt in range(5):
                        ps_ck = cpsum.tile([128, CW], f32, tag="ck")
                        for kt in range(KB):
                            nc.tensor.matmul(
                                ps_ck,
                                wkv_sb[:, kt, 128 * dt : 128 * (dt + 1)],
                                hid_sb[:, kt, cs],
                                start=(kt == 0),
                                stop=(kt == KB - 1),
                            )
                        ckb = cwork.tile([128, CW], bf16, tag="ckb")
                        nc.vector.tensor_copy(ckb, ps_ck)
                        nc.sync.dma_start(ckv_spill[:, dt, cs], ckb)
            wkvp_ctx.__exit__(None, None, None)
            hidp_ctx.__exit__(None, None, None)

            # ============ late constants + persistent attention tensors ============
            with (
                tc.tile_pool(name="consts", bufs=1) as consts,
                tc.tile_pool(name="resid", bufs=1) as resid,
            ):
                kvln_sb = consts.tile([1, KVL], f32r)
                nc.sync.dma_start(kvln_sb, kvln_d.ap())
                wukt_sb = consts.tile([128, H_PER_CORE, 4, NOPE], f32r)
                nc.sync.dma_start(
                    wukt_sb, wukt_d.ap().rearrange("h (lt p) n -> p h lt n", p=128)
                )
                wuv2_sb = consts.tile([128, 4, H_PER_CORE * VD], f32r)
                nc.sync.dma_start(
                    wuv2_sb, wuv2_d.ap().rearrange("(lt p) v -> p lt v", p=128)
                )
                cos2_sb = consts.tile([128, S], f32)
                nc.sync.dma_start(cos2_sb, cos2_d.ap())
                sin2n_sb = consts.tile([128, S], f32)
                nc.sync.dma_start(sin2n_sb, sin2n_d.ap())
                swapp_sb = consts.tile([128, 128], f32r)
                nc.sync.dma_start(swapp_sb, swapp_d.ap())
                swappb_sb = consts.tile([128, 128], bf16)
                nc.sync.dma_start(swappb_sb, swappb_d.ap())
                maskt_sb = consts.tile([128, 128], bf16)
                nc.sync.dma_start(maskt_sb, maskt_d.ap())

                kpe = resid.tile([128, S], f32r)          # roped k_peT (2 head copies)
                kn_sb = resid.tile([128, H_PER_CORE, S], f32r)  # per-head k_nopeT
                v_sb = resid.tile([128, KB, H_PER_CORE * VD], bf16)  # V seq-major
                ctxa = resid.tile([128, H_PER_CORE, S], f32r)
                wo_sb = resid.tile([128, H_PER_CORE, HID], f32r)
                nc.sync.dma_start(
                    wo_sb, wo_t.ap().rearrange("(h p) m -> p h m", p=128)
                )


                # ===== Phase K: latent rms + per-head K/V materialization =====
                # Two-stage pipeline: rms chain of chunk c overlaps the
                # materialization matmuls of chunk c-1 on the PE.
                with (
                    tc.tile_pool(name="kwork", bufs=2) as kwork,
                    tc.tile_pool(name="kpsum", bufs=2, space="PSUM") as kpsum,
                    tc.tile_pool(name="kpsum1", bufs=1, space="PSUM") as kpsum1,
                ):
                    ksn_tiles = [None] * NC_
                    ck_tiles = [None] * NC_
                    rk_tiles = [None] * NC_

                    def emit_rms_a(c):
                        cs = slice(CW * c, CW * (c + 1))
                        ck = kwork.tile([128, 5, CW], bf16, tag="ck",
                                        name=f"ck{c}")
                        nc.sync.dma_start(ck, ckv_spill[:, :, cs])
                        ps_ssqk = kpsum1.tile([1, CW], f32, tag="ssqk")
                        for j in range(4):
                            sqk = kwork.tile([128, CW], f32r, tag="sqk")
                            nc.scalar.activation(
                                sqk, ck[:, j, :], AF.Square
                            )
                            nc.tensor.matmul(
                                ps_ssqk, ones_p, sqk, start=(j == 0), stop=(j == 3)
                            )
                        rk = kwork.tile([1, CW], f32, tag="rk")
                        nc.scalar.activation(
                            rk, ps_ssqk, AF.Sqrt, scale=1.0 / KVL, bias=eps_sb
                        )
                        nc.vector.reciprocal_approx_fast(out=rk, in_=rk)
                        rk_r = kwork.tile([1, CW], f32r, tag="rkr")
                        nc.vector.tensor_copy(rk_r, rk)
                        ck_tiles[c] = ck
                        rk_tiles[c] = rk_r

                    def emit_rms_b(c):
                        ck = ck_tiles[c]
                        rk_r = rk_tiles[c]
                        ksn_c = kwork.tile([128, 4, CW], f32r, tag="ksn",
                                           name=f"ksn{c}")
                        for j in range(4):
                            ps_b = kpsum1.tile([128, CW], f32, tag="bc")
                            nc.tensor.matmul(
                                ps_b,
                                kvln_sb[0:1, 128 * j : 128 * (j + 1)],
                                rk_r,
                                start=True,
                                stop=True,
                            )
                            nc.vector.tensor_tensor(
                                ksn_c[:, j, :], ck[:, j, :], ps_b,
                                OP.mult
                            )
                        ksn_tiles[c] = ksn_c

                    def emit_mat(c):
                        cs = slice(CW * c, CW * (c + 1))
                        ksn_c = ksn_tiles[c]
                        ck = ck_tiles[c]
                        for h in range(H_PER_CORE):
                            ps_k = kpsum.tile([128, CW], f32, tag="kn")
                            for lt in range(4):
                                nc.tensor.matmul(
                                    ps_k,
                                    wukt_sb[:, h, lt, :],
                                    ksn_c[:, lt, :],
                                    start=(lt == 0),
                                    stop=(lt == 3),
                                )
                            nc.vector.tensor_copy(kn_sb[:, h, cs], ps_k)
                        for b in range(4):
                            ps_v = kpsum.tile([128, H_PER_CORE * VD], f32, tag="v")
                            for lt in range(4):
                                nc.tensor.matmul(
                                    ps_v,
                                    ksn_c[:, lt, 128 * b : 128 * (b + 1)],
                                    wuv2_sb[:, lt, :],
                                    start=(lt == 0),
                                    stop=(lt == 3),
                                )
                            nc.vector.tensor_copy(v_sb[:, 4 * c + b, :], ps_v)
                        # k_pe rope (both 64-row copies at once)
                        ps_sw = kpsum1.tile([128, CW], f32, tag="sw")
                        nc.tensor.matmul(
                            ps_sw, swappb_sb, ck[:, 4, :], start=True, stop=True
                        )
                        t1 = kwork.tile([128, CW], f32, tag="t1")
                        nc.vector.tensor_tensor(
                            t1, ck[:, 4, :], cos2_sb[:, cs], OP.mult
                        )
                        t2 = kwork.tile([128, CW], f32, tag="t2")
                        nc.vector.tensor_tensor(t2, ps_sw, sin2n_sb[:, cs], OP.mult)
                        nc.vector.tensor_tensor(kpe[:, cs], t1, t2, OP.add)

                    emit_rms_a(0)
                    emit_rms_b(0)
                    for c in range(1, NC_):
                        emit_rms_a(c)
                        emit_mat(c - 1)
                        emit_rms_b(c)
                    emit_mat(NC_ - 1)

                bres_ctx = tc.tile_pool(name="bres", bufs=1)
                bres = bres_ctx.__enter__()
                qtr = bres.tile([128, 3, S], f32r)      # post-gather q (r_q folded)
                qspe = bres.tile([128, S], f32r)        # roped q_peT
                wqb_sb = bres.tile([128, NLT, 384], bf16)
                nc.sync.dma_start(
                    wqb_sb,
                    wqb_t.ap().rearrange("(li p) m -> p li m", p=128),
                )

                # ===== Phase B: post-gather wq_b + q rope =====
                with (
                    tc.tile_pool(name="bqp", bufs=1) as bqp,
                    tc.tile_pool(name="bwork", bufs=2) as bwork,
                    tc.tile_pool(name="bpsum", bufs=2, space="PSUM") as bpsum,
                    tc.tile_pool(name="bmisc", bufs=1, space="PSUM") as bmisc,
                ):
                    qdn_all = bqp.tile([128, NLT, N_CORES, SHW], bf16)
                    qdn_src = qdn_full.rearrange(
                        "(r li p) s -> p li r s", p=128, li=NLT
                    )
                    for li in range(NLT):
                        nc.gpsimd.dma_start(
                            qdn_all[:, li, :, :], qdn_src[:, li, :, :]
                        )
                    for c in range(NC_):
                        cs = slice(CW * c, CW * (c + 1))
                        for dt in range(3):
                            ps_qt = bpsum.tile([128, CW], f32, tag="qt")
                            for li in range(NLT):
                                nc.tensor.matmul(
                                    ps_qt,
                                    wqb_sb[:, li, 128 * dt : 128 * (dt + 1)],
                                    qdn_all[:, li, 2 * c : 2 * c + 2, :],
                                    start=(li == 0),
                                    stop=(li == NLT - 1),
                                )
                            nc.vector.tensor_copy(qtr[:, dt, cs], ps_qt)
                        # rope q_pe (both heads stacked)
                        ps_sw = bmisc.tile([128, CW], f32, tag="misc",
                                           name="ps_swq")
                        nc.tensor.matmul(
                            ps_sw, swapp_sb, qtr[:, 2, cs], start=True, stop=True
                        )
                        t1 = bwork.tile([128, CW], f32, tag="t1")
                        nc.vector.tensor_tensor(
                            t1, qtr[:, 2, cs].bitcast(f32), cos2_sb[:, cs], OP.mult
                        )
                        t2 = bwork.tile([128, CW], f32, tag="t2")
                        nc.vector.tensor_tensor(t2, ps_sw, sin2n_sb[:, cs], OP.mult)
                        nc.vector.tensor_tensor(qspe[:, cs], t1, t2, OP.add)

                # ================= Phase A: attention =================
                with (
                    tc.tile_pool(name="accp", bufs=2) as accp,
                    tc.tile_pool(name="attp", bufs=3) as attp,
                    tc.tile_pool(name="aacc", bufs=1, space="PSUM") as aacc,
                    tc.tile_pool(name="ascore", bufs=2, space="PSUM") as ascore,
                    tc.tile_pool(name="ascr1", bufs=2, space="PSUM") as ascr1,
                ):
                    pending_epilogue = [None]

                    for c in range(NC_):
                        cs = slice(CW * c, CW * (c + 1))
                        nj = 4 * c + 4
                        ps_ctx = [
                            aacc.tile([128, CW], f32, tag=f"ctx{h}",
                                      name=f"ps_ctx{h}")
                            for h in range(H_PER_CORE)
                        ]
                        dacc = [
                            accp.tile([128, CW], f32, tag=f"dacc{h}",
                                      name=f"dacc{h}")
                            for h in range(H_PER_CORE)
                        ]
                        # software pipeline: scores(j) on PE, then ctx(j-1);
                        # exp(j) on ACT overlaps ctx(j-1)+scores(j+1) on PE.
                        # Softmax denominators accumulate on DVE into dacc and
                        # reduce across partitions on gpsimd -- no PE work.
                        atts = [[None, None] for _ in range(nj)]
                        offs = [0 if j < 4 * c else 128 * (j - 4 * c)
                                for j in range(nj)]

                        def emit_scores(j, c=c, nj=nj, offs=offs, atts=atts,
                                        dacc=dacc):
                            off = offs[j]
                            q0 = CW * c + off
                            qs = slice(q0, CW * (c + 1))
                            for h in range(H_PER_CORE):
                                pool = ascore if h == 0 else ascr1
                                ps_s = pool.tile([128, CW], f32, tag=f"sc{h}",
                                                 name=f"ps_s{h}")
                                nc.tensor.matmul(
                                    ps_s[:, off:],
                                    kn_sb[:, h, 128 * j : 128 * (j + 1)],
                                    qtr[:, h, qs],
                                    start=True,
                                    stop=False,
                                )
                                nc.tensor.matmul(
                                    ps_s[:, off:],
                                    kpe[64 * h : 64 * (h + 1),
                                        128 * j : 128 * (j + 1)],
                                    qspe[64 * h : 64 * (h + 1), qs],
                                    start=False,
                                    stop=True,
                                )
                                att = attp.tile([128, CW], bf16, tag=f"att{h}",
                                                name=f"att{h}")
                                nc.scalar.activation(
                                    att[:, off:], ps_s[:, off:], AF.Exp, scale=SCALE
                                )
                                if j >= 4 * c:
                                    nc.vector.tensor_tensor(
                                        att[:, off : off + 128],
                                        att[:, off : off + 128],
                                        maskt_sb,
                                        OP.mult,
                                    )
                                if j == 0:
                                    nc.vector.tensor_copy(dacc[h], att)
                                else:
                                    nc.vector.tensor_tensor(
                                        dacc[h][:, off:],
                                        dacc[h][:, off:],
                                        att[:, off:],
                                        OP.add,
                                    )
                                atts[j][h] = att

                        def emit_ctx(j, c=c, nj=nj, offs=offs, atts=atts,
                                     ps_ctx=ps_ctx):
                            off = offs[j]
                            for h in range(H_PER_CORE):
                                nc.tensor.matmul(
                                    ps_ctx[h][:, off:],
                                    v_sb[:, j, VD * h : VD * (h + 1)],
                                    atts[j][h][:, off:],
                                    start=(j == 0),
                                    stop=(j == nj - 1),
                                )

                        emit_scores(0)
                        emit_scores(1)
                        if pending_epilogue[0] is not None:
                            pending_epilogue[0]()
                        emit_ctx(0)
                        for j in range(2, nj):
                            emit_scores(j)
                            emit_ctx(j - 1)
                        emit_ctx(nj - 1)
                        for h in range(H_PER_CORE):
                            nc.gpsimd.partition_all_reduce(
                                dacc[h], dacc[h], channels=128,
                                reduce_op=bass_isa.ReduceOp.add,
                            )
                            nc.vector.reciprocal_approx_fast(
                                out=dacc[h], in_=dacc[h]
                            )

                        def epilogue(c=c, cs=cs, ps_ctx=ps_ctx, dacc=dacc):
                            for h in range(H_PER_CORE):
                                nc.vector.tensor_tensor(
                                    ctxa[:, h, cs], ps_ctx[h], dacc[h], OP.mult
                                )

                        pending_epilogue[0] = epilogue
                    pending_epilogue[0]()

                # ================= Phase W: output projection =================
                # out writes batched 4 row-blocks per DMA: 64 small DMAs would
                # saturate the SP sequencer's descriptor generation and pace
                # the PE.
                with (
                    tc.tile_pool(name="obp", bufs=2) as obp,
                    tc.tile_pool(name="wpsum", bufs=2, space="PSUM") as wpsum,
                ):
                    for c in range(NC_):
                        cs = slice(CW * c, CW * (c + 1))
                        for hg in range(4):
                            ob4 = obp.tile([128, 4, CW], bf16, tag="ob4")
                            for hi in range(4):
                                ht = 4 * hg + hi
                                ps_o = wpsum.tile([128, CW], f32, tag="o")
                                for h in range(H_PER_CORE):
                                    nc.tensor.matmul(
                                        ps_o,
                                        wo_sb[:, h, 128 * ht : 128 * (ht + 1)],
                                        ctxa[:, h, cs],
                                        start=(h == 0),
                                        stop=(h == H_PER_CORE - 1),
                                    )
                                nc.vector.tensor_copy(ob4[:, hi, :], ps_o)
                            nc.sync.dma_start(
                                out_t.ap()[512 * hg : 512 * (hg + 1), cs]
                                .rearrange("(ht p) s -> p ht s", p=128),
                                ob4,
                            )
                bres_ctx.__exit__(None, None, None)

    nc.finalize()
    return nc


_PROGRAM = None


def _get_program():
    global _PROGRAM
    if _PROGRAM is None:
        _PROGRAM = _build_program()
    return _PROGRAM


def _host_inputs(hidden_states, position_ids, wq_a, q_a_ln_w, wq_b, wkv_a,
                 kv_a_ln_w, wkv_b, wo):
    """Build the 8 per-core input maps."""
    hs = np.asarray(hidden_states, np.float32)[0]          # [S, HID]
    pos = np.asarray(position_ids)[0].astype(np.int64)     # [S]

    # rope tables (fp32, matching the reference)
    inv_freq = (1.0 / (THETA ** (np.arange(0, ROPE, 2, dtype=np.float32) / ROPE))).astype(np.float32)
    t = pos.astype(np.float32)
    freqs = np.outer(t, inv_freq).astype(np.float32)       # [S, 32]
    emb = np.concatenate([freqs, freqs], -1)               # [S, 64]
    cos = np.cos(emb).astype(np.float32)
    sin = np.sin(emb).astype(np.float32)
    cosT = np.ascontiguousarray(cos.T)                     # [64, S]
    sinT = np.ascontiguousarray(sin.T)
    sinTn = sinT.copy()
    sinTn[:32] = -sinTn[:32]                               # fold rotate_half sign
    cos2 = np.concatenate([cosT, cosT], 0)                 # [128, S]
    sin2n = np.concatenate([sinTn, sinTn], 0)

    perm = np.concatenate([np.arange(0, ROPE, 2), np.arange(1, ROPE, 2)])  # interleave

    # swap-halves permutation matrix (two independent 64 blocks)
    swapp = np.zeros((128, 128), np.float32)
    for m in range(128):
        base = (m // 64) * 64
        i = m % 64
        swapp[base + (i + 32) % 64, m] = 1.0

    maskt = np.triu(np.ones((128, 128), np.float32))

    wq_b = np.asarray(wq_b, np.float32) * np.asarray(q_a_ln_w, np.float32)[None, :]
    kvb = np.asarray(wkv_b, np.float32).reshape(16, NOPE + VD, KVL)
    wkv_a = np.asarray(wkv_a, np.float32)
    wkv_rows = np.concatenate(
        [wkv_a[:KVL], wkv_a[KVL:][perm], wkv_a[KVL:][perm]], 0
    )                                                      # [640, HID]

    hid_T = np.ascontiguousarray(hs.T)                     # [HID, S]
    shared = {
        "hid_t": _bf16(hid_T),
        "wqa_t": _bf16(np.asarray(wq_a, np.float32).T),
        "wkv_t": _bf16(wkv_rows.T),
        "kvln": _tf32_rne(np.asarray(kv_a_ln_w, np.float32)[None, :]),
        "cos2": cos2, "sin2n": sin2n,
        "swapp": _tf32_rne(swapp), "swappb": _bf16(swapp),
        "maskt": _bf16(maskt),
    }

    wo = np.asarray(wo, np.float32)
    in_maps = []
    for core in range(N_CORES):
        h0 = H_PER_CORE * core
        blocks = []
        pe_rows = []
        for h in (h0, h0 + 1):
            blk = wq_b[192 * h : 192 * (h + 1)]
            blocks.append(blk[:NOPE])
            pe_rows.append(blk[NOPE:][perm])
        wqb_re = np.concatenate(blocks + pe_rows, 0)       # [384, QLR]
        wukt = np.stack(
            [np.ascontiguousarray(kvb[h, :NOPE, :].T) for h in (h0, h0 + 1)]
        )                                                  # [2, 512, 128]
        wuv2 = np.concatenate(
            [kvb[h, NOPE:, :].T for h in (h0, h0 + 1)], axis=1
        )                                                  # [512, 256]
        wo_c = np.ascontiguousarray(wo[:, VD * h0 : VD * (h0 + 2)].T)   # [256, HID]
        in_maps.append({
            **shared,
            "hid_own": _bf16(hid_T[:, SHW * core : SHW * (core + 1)]),
            "wqb_t": _bf16(wqb_re.T),
            "wukt": _tf32_rne(wukt),
            "wuv2": _tf32_rne(np.ascontiguousarray(wuv2)),
            "wo_t": _tf32_rne(wo_c),
        })
    return in_maps


def kernel(**inputs):
    from concourse.bass_utils import run_bass_kernel_spmd

    nc = _get_program()
    in_maps = _host_inputs(**inputs)
    res = run_bass_kernel_spmd(nc, in_maps, core_ids=list(range(N_CORES)))
    acc = None
    for r in res.results:
        o = np.asarray(r["out_t"], dtype=np.float32)
        acc = o if acc is None else acc + o
    out = np.ascontiguousarray(acc.T)[None]                # [1, S, HID]
    return out.astype(np.float32)


# revision 49
# speedup vs baseline: 1.2414x; 1.0269x over previous
"""MLA (DeepSeek-style multi-head latent attention) forward on 8 TRN2 NeuronCores.

Sharding: the q_down projection (the largest replicated GEMM) is sharded over
sequence (each core computes its own 256 rows, normalized, bf16) and
AllGathered on-device while the still-replicated ckv projection and the K/V
materialization run on the tensor engine — the collective hides behind ~93us
of PE work. Attention and the output projection are tensor-parallel over
heads (16 heads -> 2 per core); partial wo outputs are summed on host.

Device layout is "feature-major" (features on SBUF partitions, sequence on the
free dim) throughout. Attention uses the prefill-optimal NON-absorbed form:
per-head K (128-dim nope) and V (128-dim) are materialized from the shared
latent once, so scores contract over 192 dims (not 576) and ctx over 128
(not 512). Scores come out k-major ([k, q]); softmax normalization over k is
done with ones-matmuls on the tensor engine.

The projections run with bf16 inputs and weights (half the DMA, same PE rate
as fp32r at free-dim >= 256), accumulating in fp32 PSUM. Attention operands
stay float32r (TF32). wo partials return in bf16 and are summed in fp32 on
host.

Pipeline per core (S=2048; 4 seq-chunks of 512 for full-S phases):
  L:  local q_down shard: q_downT[:, own 256 cols] = wq_a.T^T @ hid_ownT,
      software-pipelined li-groups (sum-of-squares via ACT Square straight
      from PSUM + ones-matmul trail one group behind); rms fold: shard is
      normalized (r_q broadcast via gpsimd) and rounded to bf16; DMA to DRAM
      -> AllGather across the 8 cores.
  C:  ckvT = wkv_a'.T^T @ hidT replicated (RoPE interleave baked into the pe
      rows of wkv_a) -> DRAM spill. Runs while the AllGather flies.
  K:  two-stage pipeline per chunk: latent rms (kv ln folded into the
      broadcast matmul), then per-head k_nopeT (feature-major) and V
      (seq-major, both heads side by side) for the previous chunk; RoPE k_pe.
  B:  post-gather wq_b: qT (all heads' rows for this core) from the gathered
      normalized q_down, PSUM-accumulated over the 12 l-tiles; RoPE q_pe.
  A:  per k-block, software-pipelined over both heads: scoresT -> exp (no
      max subtraction needed: |score*scale| <= ~4) -> causal mask on the
      diagonal block (suffix-sliced matmuls skip fully-masked columns) ->
      ctxT + softmax denominator accumulation in PSUM; per-head epilogues
      deferred into the next chunk's score stream.
  W:  wo partial matmul -> bf16 DRAM outT.
Host: sum the 8 partial outT in fp32, transpose -> [1, S, HID].
"""

import numpy as np

S = 2048
HID = 2048
QLR = 1536
H_PER_CORE = 2
N_CORES = 8
NOPE = 128
ROPE = 64
VD = 128
KVL = 512
EPS = 1e-6
THETA = 10000.0
SCALE = float((NOPE + ROPE) ** -0.5)
NC_ = 4            # seq chunks
CW = 512           # chunk width
SHW = S // N_CORES  # 256-wide local shard
KB = S // 128      # 16 k-blocks
NLT = QLR // 128   # 12 l-tiles


def _tf32_rne(a):
    a = np.ascontiguousarray(a, dtype=np.float32)
    u = a.view(np.uint32).astype(np.uint64)
    u = (u + 0xFFF + ((u >> 13) & 1)) & 0xFFFFE000
    return u.astype(np.uint32).view(np.float32)


def _bf16(a):
    import ml_dtypes
    return np.ascontiguousarray(np.asarray(a, np.float32)).astype(ml_dtypes.bfloat16)


def _build_program():
    import concourse.mybir as mybir
    import concourse.tile as tile
    import concourse.bass_isa as bass_isa
    from concourse import bacc

    f32 = mybir.dt.float32
    f32r = mybir.dt.float32r
    bf16 = mybir.dt.bfloat16
    AF = mybir.ActivationFunctionType
    OP = mybir.AluOpType

    nc = bacc.Bacc("TRN2", target_bir_lowering=False, num_devices=N_CORES)

    hid_own = nc.dram_tensor("hid_own", [HID, SHW], bf16, kind="ExternalInput")
    hid_t = nc.dram_tensor("hid_t", [HID, S], bf16, kind="ExternalInput")
    wqa_t = nc.dram_tensor("wqa_t", [HID, QLR], bf16, kind="ExternalInput")
    wqb_t = nc.dram_tensor("wqb_t", [QLR, 384], bf16, kind="ExternalInput")
    wkv_t = nc.dram_tensor("wkv_t", [HID, 640], bf16, kind="ExternalInput")
    kvln_d = nc.dram_tensor("kvln", [1, KVL], f32r, kind="ExternalInput")
    wukt_d = nc.dram_tensor("wukt", [H_PER_CORE, KVL, NOPE], f32r,
                            kind="ExternalInput")
    wuv2_d = nc.dram_tensor("wuv2", [KVL, H_PER_CORE * VD], f32r,
                            kind="ExternalInput")
    wo_t = nc.dram_tensor("wo_t", [H_PER_CORE * VD, HID], f32r, kind="ExternalInput")
    cos2_d = nc.dram_tensor("cos2", [128, S], f32, kind="ExternalInput")
    sin2n_d = nc.dram_tensor("sin2n", [128, S], f32, kind="ExternalInput")
    swapp_d = nc.dram_tensor("swapp", [128, 128], f32r, kind="ExternalInput")
    swappb_d = nc.dram_tensor("swappb", [128, 128], bf16, kind="ExternalInput")
    maskt_d = nc.dram_tensor("maskt", [128, 128], bf16, kind="ExternalInput")
    out_t = nc.dram_tensor("out_t", [HID, S], bf16, kind="ExternalOutput")

    with tile.TileContext(nc) as tc:
        with (
            tc.tile_pool(name="stats", bufs=1) as stats,
            tc.tile_pool(name="dram", bufs=1, space="DRAM") as dram,
        ):
            ones_p = stats.tile([128, 1], f32r)
            nc.vector.memset(ones_p.bitcast(f32), 1.0)
            ones_row = stats.tile([1, 128], f32r)
            nc.vector.memset(ones_row.bitcast(f32), 1.0)
            eps_sb = stats.tile([1, 1], f32)
            nc.vector.memset(eps_sb, EPS)

            ckv_spill = dram.tile([128, 5, S], bf16)
            qdn_shard = dram.tile([QLR, SHW], bf16)
            qdn_full = dram.tile([N_CORES * QLR, SHW], bf16,
                                 addr_space="Shared")

            # ========== Phase L: local q_down shard -> AllGather ==========
            # hid / wkv for phase C are DMA'd up front: once the AllGather
            # launches it monopolizes the DMA engines, so everything phase C
            # needs must already be on-chip.
            hidp_ctx = tc.tile_pool(name="hidp", bufs=1)
            hidp = hidp_ctx.__enter__()
            wkvp_ctx = tc.tile_pool(name="wkvp", bufs=1)
            wkvp = wkvp_ctx.__enter__()
            with (
                tc.tile_pool(name="hidop", bufs=1) as hidop,
                tc.tile_pool(name="wqap", bufs=1) as wqap,
                tc.tile_pool(name="lwork", bufs=2) as lwork,
                tc.tile_pool(name="lbig", bufs=1) as lbig,
                tc.tile_pool(name="lpsum", bufs=2, space="PSUM") as lpsum,
                tc.tile_pool(name="lpsum1", bufs=1, space="PSUM") as lpsum1,
            ):
                hid_own_sb = hidop.tile([128, KB, SHW], bf16)
                for kt in range(KB):
                    nc.sync.dma_start(
                        hid_own_sb[:, kt, :],
                        hid_own.ap()[128 * kt : 128 * (kt + 1), :],
                    )
                wqa_sb = wqap.tile([128, KB, QLR], bf16)
                for lg in range(3):
                    nc.sync.dma_start(
                        wqa_sb[:, :, 512 * lg : 512 * (lg + 1)],
                        wqa_t.ap()[:, 512 * lg : 512 * (lg + 1)].rearrange(
                            "(kt p) m -> p kt m", p=128
                        ),
                    )
                wkv_sb = wkvp.tile([128, KB, 640], bf16, tag="wkv")
                for dt in range(5):
                    nc.sync.dma_start(
                        wkv_sb[:, :, 128 * dt : 128 * (dt + 1)],
                        wkv_t.ap()[:, 128 * dt : 128 * (dt + 1)].rearrange(
                            "(kt p) m -> p kt m", p=128
                        ),
                    )
                hid_sb = hidp.tile([128, KB, S], bf16)
                for kt in range(KB):
                    nc.sync.dma_start(
                        hid_sb[:, kt, :], hid_t.ap()[128 * kt : 128 * (kt + 1), :]
                    )
                qdraw = lbig.tile([128, NLT, SHW], f32)
                qdn_sb = lbig.tile([128, NLT, SHW], bf16)
                ssql = stats.tile([1, SHW], f32)
                rqbl = stats.tile([128, SHW], f32)
                ps_ssq = lpsum1.tile([1, SHW], f32, tag="ssq")

                prev_qd = [None]

                def emit_ssq(li):
                    sq = lwork.tile([128, SHW], f32r, tag="sq")
                    nc.scalar.activation(sq, prev_qd[0], AF.Square)
                    nc.tensor.matmul(
                        ps_ssq, ones_p, sq, start=(li == 0), stop=(li == NLT - 1)
                    )

                for li in range(NLT):
                    ps_qd = lpsum.tile([128, SHW], f32, tag="qd",
                                       name=f"ps_qd{li % 2}")
                    for kt in range(KB):
                        nc.tensor.matmul(
                            ps_qd,
                            wqa_sb[:, kt, 128 * li : 128 * (li + 1)],
                            hid_own_sb[:, kt, :],
                            start=(kt == 0),
                            stop=(kt == KB - 1),
                        )
                    if li > 0:
                        emit_ssq(li - 1)
                    nc.vector.tensor_copy(qdraw[:, li, :], ps_qd)
                    prev_qd[0] = ps_qd
                emit_ssq(NLT - 1)
                nc.scalar.activation(
                    ssql, ps_ssq, AF.Sqrt, scale=1.0 / QLR, bias=eps_sb
                )
                nc.vector.reciprocal_approx_fast(out=ssql, in_=ssql)
                nc.gpsimd.partition_broadcast(rqbl, ssql, channels=128)
                for li in range(NLT):
                    nc.vector.tensor_tensor(
                        qdn_sb[:, li, :], qdraw[:, li, :], rqbl, OP.mult
                    )
                nc.gpsimd.dma_start(
                    qdn_shard.rearrange("(li p) s -> p li s", p=128),
                    qdn_sb,
                )
                nc.gpsimd.collective_compute(
                    "AllGather",
                    mybir.AluOpType.bypass,
                    replica_groups=[list(range(N_CORES))],
                    ins=[qdn_shard.opt()],
                    outs=[qdn_full.opt()],
                )

            # ========== Phase C: replicated ckvT (overlaps the AllGather) =====
            # bf16 spill: the AllGather is saturating the DMA engines, so the
            # spill round-trip is kept as small as possible.
            with (
                tc.tile_pool(name="cwork", bufs=3) as cwork,
                tc.tile_pool(name="cpsum", bufs=2, space="PSUM") as cpsum,
            ):
                for c in range(NC_):
                    cs = slice(CW * c, CW * (c + 1))
                    for dt in range(5):
                        ps_ck = cpsum.tile([128, CW], f32, tag="ck")
                        for kt in range(KB):
                            nc.tensor.matmul(
                                ps_ck,
                                wkv_sb[:, kt, 128 * dt : 128 * (dt + 1)],
                                hid_sb[:, kt, cs],
                                start=(kt == 0),
                                stop=(kt == KB - 1),
                            )
                        ckb = cwork.tile([128, CW], bf16, tag="ckb")
                        nc.vector.tensor_copy(ckb, ps_ck)
                        nc.scalar.dma_start(ckv_spill[:, dt, cs], ckb)
            wkvp_ctx.__exit__(None, None, None)
            hidp_ctx.__exit__(None, None, None)

            # ============ late constants + persistent attention tensors ============
            with (
                tc.tile_pool(name="consts", bufs=1) as consts,
                tc.tile_pool(name="resid", bufs=1) as resid,
            ):
                kvln_sb = consts.tile([1, KVL], f32r)
                nc.sync.dma_start(kvln_sb, kvln_d.ap())
                wukt_sb = consts.tile([128, H_PER_CORE, 4, NOPE], f32r)
                nc.sync.dma_start(
                    wukt_sb, wukt_d.ap().rearrange("h (lt p) n -> p h lt n", p=128)
                )
                wuv2_sb = consts.tile([128, 4, H_PER_CORE * VD], f32r)
                nc.sync.dma_start(
                    wuv2_sb, wuv2_d.ap().rearrange("(lt p) v -> p lt v", p=128)
                )
                cos2_sb = consts.tile([128, S], f32)
                nc.sync.dma_start(cos2_sb, cos2_d.ap())
                sin2n_sb = consts.tile([128, S], f32)
                nc.sync.dma_start(sin2n_sb, sin2n_d.ap())
                swapp_sb = consts.tile([128, 128], f32r)
                nc.sync.dma_start(swapp_sb, swapp_d.ap())
                swappb_sb = consts.tile([128, 128], bf16)
                nc.sync.dma_start(swappb_sb, swappb_d.ap())
                maskt_sb = consts.tile([128, 128], bf16)
                nc.sync.dma_start(maskt_sb, maskt_d.ap())

                kpe = resid.tile([128, S], f32r)          # roped k_peT (2 head copies)
                kn_sb = resid.tile([128, H_PER_CORE, S], f32r)  # per-head k_nopeT
                v_sb = resid.tile([128, KB, H_PER_CORE * VD], bf16)  # V seq-major
                ctxa = resid.tile([128, H_PER_CORE, S], f32r)
                wo_sb = resid.tile([128, H_PER_CORE, HID], f32r)
                nc.sync.dma_start(
                    wo_sb, wo_t.ap().rearrange("(h p) m -> p h m", p=128)
                )


                # ===== Phase K: latent rms + per-head K/V materialization =====
                # Two-stage pipeline: rms chain of chunk c overlaps the
                # materialization matmuls of chunk c-1 on the PE.
                with (
                    tc.tile_pool(name="kwork", bufs=2) as kwork,
                    tc.tile_pool(name="kpsum", bufs=2, space="PSUM") as kpsum,
                    tc.tile_pool(name="kpsum1", bufs=1, space="PSUM") as kpsum1,
                ):
                    ksn_tiles = [None] * NC_
                    ck_tiles = [None] * NC_
                    rk_tiles = [None] * NC_

                    def emit_rms_a(c):
                        cs = slice(CW * c, CW * (c + 1))
                        ck = kwork.tile([128, 5, CW], bf16, tag="ck",
                                        name=f"ck{c}")
                        nc.sync.dma_start(ck, ckv_spill[:, :, cs])
                        ps_ssqk = kpsum1.tile([1, CW], f32, tag="ssqk")
                        for j in range(4):
                            sqk = kwork.tile([128, CW], f32r, tag="sqk")
                            nc.scalar.activation(
                                sqk, ck[:, j, :], AF.Square
                            )
                            nc.tensor.matmul(
                                ps_ssqk, ones_p, sqk, start=(j == 0), stop=(j == 3)
                            )
                        rk = kwork.tile([1, CW], f32, tag="rk")
                        nc.scalar.activation(
                            rk, ps_ssqk, AF.Sqrt, scale=1.0 / KVL, bias=eps_sb
                        )
                        nc.vector.reciprocal_approx_fast(out=rk, in_=rk)
                        rk_r = kwork.tile([1, CW], f32r, tag="rkr")
                        nc.vector.tensor_copy(rk_r, rk)
                        ck_tiles[c] = ck
                        rk_tiles[c] = rk_r

                    def emit_rms_b(c):
                        ck = ck_tiles[c]
                        rk_r = rk_tiles[c]
                        ksn_c = kwork.tile([128, 4, CW], f32r, tag="ksn",
                                           name=f"ksn{c}")
                        for j in range(4):
                            ps_b = kpsum1.tile([128, CW], f32, tag="bc")
                            nc.tensor.matmul(
                                ps_b,
                                kvln_sb[0:1, 128 * j : 128 * (j + 1)],
                                rk_r,
                                start=True,
                                stop=True,
                            )
                            nc.vector.tensor_tensor(
                                ksn_c[:, j, :], ck[:, j, :], ps_b,
                                OP.mult
                            )
                        ksn_tiles[c] = ksn_c

                    def emit_mat(c):
                        cs = slice(CW * c, CW * (c + 1))
                        ksn_c = ksn_tiles[c]
                        ck = ck_tiles[c]
                        for h in range(H_PER_CORE):
                            ps_k = kpsum.tile([128, CW], f32, tag="kn")
                            for lt in range(4):
                                nc.tensor.matmul(
                                    ps_k,
                                    wukt_sb[:, h, lt, :],
                                    ksn_c[:, lt, :],
                                    start=(lt == 0),
                                    stop=(lt == 3),
                                )
                            nc.vector.tensor_copy(kn_sb[:, h, cs], ps_k)
                        for b in range(4):
                            ps_v = kpsum.tile([128, H_PER_CORE * VD], f32, tag="v")
                            for lt in range(4):
                                nc.tensor.matmul(
                                    ps_v,
                                    ksn_c[:, lt, 128 * b : 128 * (b + 1)],
                                    wuv2_sb[:, lt, :],
                                    start=(lt == 0),
                                    stop=(lt == 3),
                                )
                            nc.vector.tensor_copy(v_sb[:, 4 * c + b, :], ps_v)
                        # k_pe rope (both 64-row copies at once)
                        ps_sw = kpsum1.tile([128, CW], f32, tag="sw")
                        nc.tensor.matmul(
                            ps_sw, swappb_sb, ck[:, 4, :], start=True, stop=True
                        )
                        t1 = kwork.tile([128, CW], f32, tag="t1")
                        nc.vector.tensor_tensor(
                            t1, ck[:, 4, :], cos2_sb[:, cs], OP.mult
                        )
                        t2 = kwork.tile([128, CW], f32, tag="t2")
                        nc.vector.tensor_tensor(t2, ps_sw, sin2n_sb[:, cs], OP.mult)
                        nc.vector.tensor_tensor(kpe[:, cs], t1, t2, OP.add)

                    emit_rms_a(0)
                    emit_rms_b(0)
                    for c in range(1, NC_):
                        emit_rms_a(c)
                        emit_mat(c - 1)
                        emit_rms_b(c)
                    emit_mat(NC_ - 1)

                bres_ctx = tc.tile_pool(name="bres", bufs=1)
                bres = bres_ctx.__enter__()
                qtr = bres.tile([128, 3, S], f32r)      # post-gather q (r_q folded)
                qspe = bres.tile([128, S], f32r)        # roped q_peT
                wqb_sb = bres.tile([128, NLT, 384], bf16)
                nc.sync.dma_start(
                    wqb_sb,
                    wqb_t.ap().rearrange("(li p) m -> p li m", p=128),
                )

                # ===== Phase B: post-gather wq_b + q rope =====
                with (
                    tc.tile_pool(name="bqp", bufs=1) as bqp,
                    tc.tile_pool(name="bwork", bufs=2) as bwork,
                    tc.tile_pool(name="bpsum", bufs=2, space="PSUM") as bpsum,
                    tc.tile_pool(name="bmisc", bufs=1, space="PSUM") as bmisc,
                ):
                    qdn_all = bqp.tile([128, NLT, N_CORES, SHW], bf16)
                    qdn_src = qdn_full.rearrange(
                        "(r li p) s -> p li r s", p=128, li=NLT
                    )
                    for li in range(NLT):
                        nc.gpsimd.dma_start(
                            qdn_all[:, li, :, :], qdn_src[:, li, :, :]
                        )
                    for c in range(NC_):
                        cs = slice(CW * c, CW * (c + 1))
                        for dt in range(3):
                            ps_qt = bpsum.tile([128, CW], f32, tag="qt")
                            for li in range(NLT):
                                nc.tensor.matmul(
                                    ps_qt,
                                    wqb_sb[:, li, 128 * dt : 128 * (dt + 1)],
                                    qdn_all[:, li, 2 * c : 2 * c + 2, :],
                                    start=(li == 0),
                                    stop=(li == NLT - 1),
                                )
                            nc.vector.tensor_copy(qtr[:, dt, cs], ps_qt)
                        # rope q_pe (both heads stacked)
                        ps_sw = bmisc.tile([128, CW], f32, tag="misc",
                                           name="ps_swq")
                        nc.tensor.matmul(
                            ps_sw, swapp_sb, qtr[:, 2, cs], start=True, stop=True
                        )
                        t1 = bwork.tile([128, CW], f32, tag="t1")
                        nc.vector.tensor_tensor(
                            t1, qtr[:, 2, cs].bitcast(f32), cos2_sb[:, cs], OP.mult
                        )
                        t2 = bwork.tile([128, CW], f32, tag="t2")
                        nc.vector.tensor_tensor(t2, ps_sw, sin2n_sb[:, cs], OP.mult)
                        nc.vector.tensor_tensor(qspe[:, cs], t1, t2, OP.add)

                # ================= Phase A: attention =================
                with (
                    tc.tile_pool(name="accp", bufs=2) as accp,
                    tc.tile_pool(name="attp", bufs=3) as attp,
                    tc.tile_pool(name="obp", bufs=2) as obp,
                    tc.tile_pool(name="aacc", bufs=1, space="PSUM") as aacc,
                    tc.tile_pool(name="ascore", bufs=2, space="PSUM") as ascore,
                    tc.tile_pool(name="ascr1", bufs=2, space="PSUM") as ascr1,
                    tc.tile_pool(name="wpsum", bufs=2, space="PSUM") as wpsum,
                ):
                    pending_epilogue = [None]

                    def emit_w_group(c, hg):
                        cs = slice(CW * c, CW * (c + 1))
                        ob4 = obp.tile([128, 4, CW], bf16, tag="ob4")
                        for hi in range(4):
                            ht = 4 * hg + hi
                            ps_o = wpsum.tile([128, CW], f32, tag="o")
                            for h in range(H_PER_CORE):
                                nc.tensor.matmul(
                                    ps_o,
                                    wo_sb[:, h, 128 * ht : 128 * (ht + 1)],
                                    ctxa[:, h, cs],
                                    start=(h == 0),
                                    stop=(h == H_PER_CORE - 1),
                                )
                            nc.vector.tensor_copy(ob4[:, hi, :], ps_o)
                        nc.sync.dma_start(
                            out_t.ap()[512 * hg : 512 * (hg + 1), cs]
                            .rearrange("(ht p) s -> p ht s", p=128),
                            ob4,
                        )

                    for c in range(NC_):
                        cs = slice(CW * c, CW * (c + 1))
                        nj = 4 * c + 4
                        ps_ctx = [
                            aacc.tile([128, CW], f32, tag=f"ctx{h}",
                                      name=f"ps_ctx{h}")
                            for h in range(H_PER_CORE)
                        ]
                        dacc = [
                            accp.tile([128, CW], f32, tag=f"dacc{h}",
                                      name=f"dacc{h}")
                            for h in range(H_PER_CORE)
                        ]
                        # software pipeline: scores(j) on PE, then ctx(j-1);
                        # exp(j) on ACT overlaps ctx(j-1)+scores(j+1) on PE.
                        # Softmax denominators accumulate on DVE into dacc and
                        # reduce across partitions on gpsimd -- no PE work.
                        atts = [[None, None] for _ in range(nj)]
                        offs = [0 if j < 4 * c else 128 * (j - 4 * c)
                                for j in range(nj)]

                        def emit_scores(j, c=c, nj=nj, offs=offs, atts=atts,
                                        dacc=dacc):
                            off = offs[j]
                            q0 = CW * c + off
                            qs = slice(q0, CW * (c + 1))
                            for h in range(H_PER_CORE):
                                pool = ascore if h == 0 else ascr1
                                ps_s = pool.tile([128, CW], f32, tag=f"sc{h}",
                                                 name=f"ps_s{h}")
                                nc.tensor.matmul(
                                    ps_s[:, off:],
                                    kn_sb[:, h, 128 * j : 128 * (j + 1)],
                                    qtr[:, h, qs],
                                    start=True,
                                    stop=False,
                                )
                                nc.tensor.matmul(
                                    ps_s[:, off:],
                                    kpe[64 * h : 64 * (h + 1),
                                        128 * j : 128 * (j + 1)],
                                    qspe[64 * h : 64 * (h + 1), qs],
                                    start=False,
                                    stop=True,
                                )
                                att = attp.tile([128, CW], bf16, tag=f"att{h}",
                                                name=f"att{h}")
                                nc.scalar.activation(
                                    att[:, off:], ps_s[:, off:], AF.Exp, scale=SCALE
                                )
                                if j >= 4 * c:
                                    nc.vector.tensor_tensor(
                                        att[:, off : off + 128],
                                        att[:, off : off + 128],
                                        maskt_sb,
                                        OP.mult,
                                    )
                                if j == 0:
                                    nc.vector.tensor_copy(dacc[h], att)
                                else:
                                    nc.vector.tensor_tensor(
                                        dacc[h][:, off:],
                                        dacc[h][:, off:],
                                        att[:, off:],
                                        OP.add,
                                    )
                                atts[j][h] = att

                        def emit_ctx(j, c=c, nj=nj, offs=offs, atts=atts,
                                     ps_ctx=ps_ctx):
                            off = offs[j]
                            for h in range(H_PER_CORE):
                                nc.tensor.matmul(
                                    ps_ctx[h][:, off:],
                                    v_sb[:, j, VD * h : VD * (h + 1)],
                                    atts[j][h][:, off:],
                                    start=(j == 0),
                                    stop=(j == nj - 1),
                                )

                        emit_scores(0)
                        emit_scores(1)
                        if pending_epilogue[0] is not None:
                            pending_epilogue[0]()
                        emit_ctx(0)
                        pending_w = (
                            [(c - 1, hg) for hg in range(4)] if c >= 1 else []
                        )
                        for j in range(2, nj):
                            emit_scores(j)
                            emit_ctx(j - 1)
                            if pending_w and j >= 3:
                                emit_w_group(*pending_w.pop(0))
                        emit_ctx(nj - 1)
                        while pending_w:
                            emit_w_group(*pending_w.pop(0))
                        for h in range(H_PER_CORE):
                            nc.gpsimd.partition_all_reduce(
                                dacc[h], dacc[h], channels=128,
                                reduce_op=bass_isa.ReduceOp.add,
                            )
                            nc.vector.reciprocal_approx_fast(
                                out=dacc[h], in_=dacc[h]
                            )

                        def epilogue(c=c, cs=cs, ps_ctx=ps_ctx, dacc=dacc):
                            for h in range(H_PER_CORE):
                                nc.vector.tensor_tensor(
                                    ctxa[:, h, cs], ps_ctx[h], dacc[h], OP.mult
                                )

                        pending_epilogue[0] = epilogue
                    pending_epilogue[0]()
                    for hg in range(4):
                        emit_w_group(NC_ - 1, hg)
                bres_ctx.__exit__(None, None, None)

    nc.finalize()
    return nc


_PROGRAM = None


def _get_program():
    global _PROGRAM
    if _PROGRAM is None:
        _PROGRAM = _build_program()
    return _PROGRAM


def _host_inputs(hidden_states, position_ids, wq_a, q_a_ln_w, wq_b, wkv_a,
                 kv_a_ln_w, wkv_b, wo):
    """Build the 8 per-core input maps."""
    hs = np.asarray(hidden_states, np.float32)[0]          # [S, HID]
    pos = np.asarray(position_ids)[0].astype(np.int64)     # [S]

    # rope tables (fp32, matching the reference)
    inv_freq = (1.0 / (THETA ** (np.arange(0, ROPE, 2, dtype=np.float32) / ROPE))).astype(np.float32)
    t = pos.astype(np.float32)
    freqs = np.outer(t, inv_freq).astype(np.float32)       # [S, 32]
    emb = np.concatenate([freqs, freqs], -1)               # [S, 64]
    cos = np.cos(emb).astype(np.float32)
    sin = np.sin(emb).astype(np.float32)
    cosT = np.ascontiguousarray(cos.T)                     # [64, S]
    sinT = np.ascontiguousarray(sin.T)
    sinTn = sinT.copy()
    sinTn[:32] = -sinTn[:32]                               # fold rotate_half sign
    cos2 = np.concatenate([cosT, cosT], 0)                 # [128, S]
    sin2n = np.concatenate([sinTn, sinTn], 0)

    perm = np.concatenate([np.arange(0, ROPE, 2), np.arange(1, ROPE, 2)])  # interleave

    # swap-halves permutation matrix (two independent 64 blocks)
    swapp = np.zeros((128, 128), np.float32)
    for m in range(128):
        base = (m // 64) * 64
        i = m % 64
        swapp[base + (i + 32) % 64, m] = 1.0

    maskt = np.triu(np.ones((128, 128), np.float32))

    wq_b = np.asarray(wq_b, np.float32) * np.asarray(q_a_ln_w, np.float32)[None, :]
    kvb = np.asarray(wkv_b, np.float32).reshape(16, NOPE + VD, KVL)
    wkv_a = np.asarray(wkv_a, np.float32)
    wkv_rows = np.concatenate(
        [wkv_a[:KVL], wkv_a[KVL:][perm], wkv_a[KVL:][perm]], 0
    )                                                      # [640, HID]

    hid_T = np.ascontiguousarray(hs.T)                     # [HID, S]
    shared = {
        "hid_t": _bf16(hid_T),
        "wqa_t": _bf16(np.asarray(wq_a, np.float32).T),
        "wkv_t": _bf16(wkv_rows.T),
        "kvln": _tf32_rne(np.asarray(kv_a_ln_w, np.float32)[None, :]),
        "cos2": cos2, "sin2n": sin2n,
        "swapp": _tf32_rne(swapp), "swappb": _bf16(swapp),
        "maskt": _bf16(maskt),
    }

    wo = np.asarray(wo, np.float32)
    in_maps = []
    for core in range(N_CORES):
        h0 = H_PER_CORE * core
        blocks = []
        pe_rows = []
        for h in (h0, h0 + 1):
            blk = wq_b[192 * h : 192 * (h + 1)]
            blocks.append(blk[:NOPE])
            pe_rows.append(blk[NOPE:][perm])
        wqb_re = np.concatenate(blocks + pe_rows, 0)       # [384, QLR]
        wukt = np.stack(
            [np.ascontiguousarray(kvb[h, :NOPE, :].T) for h in (h0, h0 + 1)]
        )                                                  # [2, 512, 128]
        wuv2 = np.concatenate(
            [kvb[h, NOPE:, :].T for h in (h0, h0 + 1)], axis=1
        )                                                  # [512, 256]
        wo_c = np.ascontiguousarray(wo[:, VD * h0 : VD * (h0 + 2)].T)   # [256, HID]
        in_maps.append({
            **shared,
            "hid_own": _bf16(hid_T[:, SHW * core : SHW * (core + 1)]),
            "wqb_t": _bf16(wqb_re.T),
            "wukt": _tf32_rne(wukt),
            "wuv2": _tf32_rne(np.ascontiguousarray(wuv2)),
            "wo_t": _tf32_rne(wo_c),
        })
    return in_maps


def kernel(**inputs):
    from concourse.bass_utils import run_bass_kernel_spmd

    nc = _get_program()
    in_maps = _host_inputs(**inputs)
    res = run_bass_kernel_spmd(nc, in_maps, core_ids=list(range(N_CORES)))
    acc = None
    for r in res.results:
        o = np.asarray(r["out_t"], dtype=np.float32)
        acc = o if acc is None else acc + o
    out = np.ascontiguousarray(acc.T)[None]                # [1, S, HID]
    return out.astype(np.float32)
